# revision 29
# baseline (speedup 1.0000x reference)
"""DualResolutionAttention Trainium2 kernel v3 (8 NeuronCores, Bass/Tile).

Sharding: core c -> (batch b = c//4, group g = c%4); heads {2g, 2g+1} both
branches; output channel slice [128g, 128g+128) of each branch.

v3 vs v2:
- whole datapath bf16 (x input shipped bf16): LDWEIGHTS pipelines (no fp32r
  self-load tax), input DMA halves to 8MB
- attention: 3-way rotating [128,1024] score psum tiles + per-head exp ->
  scalar engine stays saturated (no scores->exp serialization)
- PV col-tiled packed: h0 -> psum partitions 0:64, h1 -> 64:128, one
  [128,1024] accumulator (2 banks)
- softmax denominators: DVE-accumulated den_acc (bf16) + ones-column matmul
  partition-reduce; no ones-column in v, no stag reshape machinery
- compress split into 4 quarter-chunks run at strip boundaries, accumulated
  in SBUF via DVE adds; cg AllGather overlaps strips 2-3
"""
import os
import sys

sys.path.insert(0, "/opt/trn_rl_repo")
os.environ.setdefault("JAX_PLATFORMS", "axon,cpu")

from contextlib import ExitStack

import numpy as np

import concourse.bass as bass
import concourse.mybir as mybir
import concourse.tile as tile
from concourse import bacc
from concourse.bass_utils import run_bass_kernel_spmd
from concourse.masks import make_identity

FP32 = mybir.dt.float32
FP32R = mybir.dt.float32r
BF16 = mybir.dt.bfloat16
AF = mybir.ActivationFunctionType
ALU = mybir.AluOpType

B, T, E = 2, 4096, 1024
LD = 512
D = 64
R = 4
Tc = T // R
NCORES = 8
GROUPS = [[0, 1, 2, 3], [4, 5, 6, 7]]
NEG = -1.0e9


def build_program():
    nc = bacc.Bacc(None, target_bir_lowering=False)

    def inp(name, shape, dt=BF16):
        return nc.declare_dram_parameter(name, list(shape), dt, isOutput=False)

    x2t = inp("x2t", [8, 128, T])            # x[b].T chunks, bf16
    wc = inp("wc", [32, 128, 128])           # compress slice lhsT chunks
    bc = inp("bc", [128, 1], FP32)
    wqkA = inp("wqkA", [4, 128, 128])        # [q_h0/8 | q_h1/8] lhsT chunks
    wqkB = inp("wqkB", [4, 128, 128])        # [k_h0 | k_h1] lhsT chunks
    bqkA = inp("bqkA", [128, 1], FP32)
    bqkB = inp("bqkB", [128, 1], FP32)
    wv = inp("wv", [4, 128, 128])            # [v_h0 | v_h1] rhs chunks
    bvr = inp("bvr", [1, 128])               # [bv_h0 | bv_h1] bias row
    wgqkA = inp("wgqkA", [4, 128, 128])
    wgqkB = inp("wgqkB", [4, 128, 128])
    bgqkA = inp("bgqkA", [128, 1], FP32)
    bgqkB = inp("bgqkB", [128, 1], FP32)
    wgv = inp("wgv", [4, 128, 128])
    bgvr = inp("bgvr", [1, 128])
    wpl = inp("wpl", [4, 128, 129])          # [w_lproj slice | u_l] chunks
    wpg = inp("wpg", [4, 128, 129])
    bple = inp("bple", [1, 129])             # [b_lproj slice, 0]
    bpge = inp("bpge", [1, 129])
    c0h = inp("c0h", [128, 1], FP32)         # 0.5 * gate const
    onesr = inp("onesr", [1, 129])           # ones row (bf16)
    onescb = inp("onescb", [128, 1])         # ones column (bf16)
    repA = inp("repA", [128, 128])           # x4 expander (even 32-blocks)
    repB = inp("repB", [128, 128])
    maskt = inp("maskt", [128, 128])         # strict lower-tri NEG
    out_loc = nc.declare_dram_parameter("out_loc", [T, 128], FP32, isOutput=True)
    out_glob = nc.declare_dram_parameter("out_glob", [T, 128], FP32,
                                         isOutput=True)

    with tile.TileContext(nc) as tc:
      with ExitStack() as top:
        dram = top.enter_context(tc.tile_pool(name="dram", bufs=1, space="DRAM"))
        const = top.enter_context(tc.tile_pool(name="const", bufs=1))
        persist = top.enter_context(tc.tile_pool(name="persist", bufs=1))

        # ---- constants
        ident_b = const.tile([128, 128], BF16, name="ident_b")
        make_identity(nc, ident_b[:])
        mask_tri = const.tile([128, 128], BF16, name="mask_tri")
        nc.sync.dma_start(out=mask_tri[:], in_=maskt[:])
        repA_sb = const.tile([128, 128], BF16, name="repA_sb")
        nc.sync.dma_start(out=repA_sb[:], in_=repA[:])
        repB_sb = const.tile([128, 128], BF16, name="repB_sb")
        nc.sync.dma_start(out=repB_sb[:], in_=repB[:])
        onescb_sb = const.tile([128, 1], BF16, name="onescb_sb")
        nc.sync.dma_start(out=onescb_sb[:], in_=onescb[:])
        ones_bf = const.tile([1, 129], BF16, name="ones_bf")
        nc.sync.dma_start(out=ones_bf[:], in_=onesr[:])
        bple_sb = const.tile([1, 129], BF16, name="bple_sb")
        nc.sync.dma_start(out=bple_sb[:], in_=bple[:])
        bpge_sb = const.tile([1, 129], BF16, name="bpge_sb")
        nc.sync.dma_start(out=bpge_sb[:], in_=bpge[:])
        bvr_sb = const.tile([1, 128], BF16, name="bvr_sb")
        nc.sync.dma_start(out=bvr_sb[:], in_=bvr[:])
        bgvr_sb = const.tile([1, 128], BF16, name="bgvr_sb")
        nc.sync.dma_start(out=bgvr_sb[:], in_=bgvr[:])
        biases = {}
        for nm, src in (("bc", bc), ("c0h", c0h),
                        ("bqkA", bqkA), ("bqkB", bqkB),
                        ("bgqkA", bgqkA), ("bgqkB", bgqkB)):
            t = const.tile([128, 1], FP32, name=f"cb_{nm}")
            nc.sync.dma_start(out=t[:], in_=src[:])
            biases[nm] = t

        # ---- persistent tensors (all bf16)
        qT_l = persist.tile([128, T], BF16, name="qT_l")
        kT_l = persist.tile([128, T], BF16, name="kT_l")
        qT_g = persist.tile([128, Tc], BF16, name="qT_g")
        kT_g = persist.tile([128, Tc], BF16, name="kT_g")
        v_sb_l = [persist.tile([128, 128], BF16, name=f"vsb{i}")
                  for i in range(32)]
        v_sb_g = [persist.tile([128, 128], BF16, name=f"vgsb{i}")
                  for i in range(8)]
        cg_all = [persist.tile([128, Tc], BF16, name=f"cg_all{i}")
                  for i in range(4)]
        cgT32 = persist.tile([128, Tc], FP32, name="cgT32")

        # DRAM bounce tiles for collectives
        cg_contrib = dram.tile([128, Tc], BF16, name="cg_contrib")
        cg_gathered = dram.tile([512, Tc], BF16, name="cg_gathered")
        attl_contrib = [dram.tile([128, 1024], BF16, name=f"alc{s}")
                        for s in range(4)]
        attl_gathered = [dram.tile([512, 1024], BF16, name=f"alg{s}")
                         for s in range(4)]
        attg_contrib = dram.tile([128, Tc], BF16, name="agc")
        attg_gathered = dram.tile([512, Tc], BF16, name="agg")

        # =========================================== phase AB: x load, qkv, v
        pab = top.enter_context(ExitStack())
        xq_pool = pab.enter_context(tc.tile_pool(name="xq_pool", bufs=1))
        xs_pool = pab.enter_context(tc.tile_pool(name="xs_pool", bufs=8))
        w_pool = pab.enter_context(tc.tile_pool(name="w_pool", bufs=1))
        wcp = pab.enter_context(tc.tile_pool(name="wc_pool", bufs=8))

        x_lo = []
        for cc in range(4):
            t = xq_pool.tile([128, T], BF16, name=f"x2t{cc}")
            nc.sync.dma_start(out=t[:], in_=x2t[cc])
            x_lo.append(t)

        wqkA_sb, wqkB_sb, wv_sb = [], [], []
        for cc in range(4):
            t = w_pool.tile([128, 128], BF16, name=f"wqkA{cc}")
            nc.sync.dma_start(out=t[:], in_=wqkA[cc])
            wqkA_sb.append(t)
            t = w_pool.tile([128, 128], BF16, name=f"wqkB{cc}")
            nc.sync.dma_start(out=t[:], in_=wqkB[cc])
            wqkB_sb.append(t)
            t = w_pool.tile([128, 128], BF16, name=f"wv{cc}")
            nc.sync.dma_start(out=t[:], in_=wv[cc])
            wv_sb.append(t)

        with ExitStack() as pqkv:
            psAB = pqkv.enter_context(
                tc.tile_pool(name="psAB", bufs=2, space="PSUM"))
            psV = pqkv.enter_context(
                tc.tile_pool(name="psV", bufs=2, space="PSUM"))
            # local q/k: packed tiles -> direct DVE copy
            for (wts, bias_ap, dst) in ((wqkA_sb, biases["bqkA"][:], qT_l),
                                        (wqkB_sb, biases["bqkB"][:], kT_l)):
                for qt in range(8):
                    ps = psAB.tile([128, 512], FP32, name="psAB_t")
                    for cc in range(4):
                        nc.tensor.matmul(
                            ps[:], wts[cc][:],
                            x_lo[cc][:, 512 * qt:512 * qt + 512],
                            start=(cc == 0), stop=(cc == 3))
                    with nc.allow_low_precision(reason="qk bf16"):
                        nc.scalar.activation(dst[:, 512 * qt:512 * qt + 512],
                                             ps[:], AF.Identity, bias=bias_ap)
            # local v: x-stationary, v_sb[t, d] direct
            for kb in range(32):
                ps = psV.tile([128, 128], FP32, name="psV_t")
                for cc in range(4):
                    nc.tensor.matmul(
                        ps[:], x_lo[cc][:, 128 * kb:128 * kb + 128],
                        wv_sb[cc][:], start=(cc == 0), stop=False)
                nc.tensor.matmul(ps[:], ones_bf[:, 0:128], bvr_sb[:],
                                 start=False, stop=True)
                with nc.allow_low_precision(reason="v bf16"):
                    nc.vector.tensor_copy(v_sb_l[kb][:], ps[:])

        # =============================================== attention machinery
        def attention_strip(pools, nkb, q0, qT, kT, v_sb):
            """One 1024-q strip, both heads; returns (hold, den_sb[2])."""
            s_pool, o_pool, p_pool, hold_pool, den_pool, dn_pool = pools
            psum_o = o_pool.tile([128, 1024], FP32, name="ps_o")
            den_acc = [den_pool.tile([128, 1024], BF16, name="den_t")
                       for _ in range(2)]
            pend = None

            def emit_pv(j, ps_list, t0):
                for h in range(2):
                    for qs in (0, 512):
                        lo = max(qs, t0)
                        hi = qs + 512
                        if lo >= hi:
                            continue
                        nc.tensor.matmul(
                            psum_o[64 * h:64 * h + 64, lo:hi],
                            v_sb[j][:, 64 * h:64 * h + 64],
                            ps_list[h][:, lo:hi],
                            start=(j == 0), stop=(j == nkb - 1),
                            skip_group_check=True)

            for j in range(nkb):
                t0 = max(0, 128 * j - q0)
                diag = 128 * j >= q0
                p2 = []
                for h in range(2):
                    ps = s_pool.tile([128, 1024], FP32, name="ps_s")
                    for qs in (0, 512):
                        if qs + 512 <= (t0 // 512) * 512:
                            continue
                        nc.tensor.matmul(
                            ps[:, qs:qs + 512],
                            kT[64 * h:64 * h + 64, 128 * j:128 * j + 128],
                            qT[64 * h:64 * h + 64, q0 + qs:q0 + qs + 512],
                            start=True, stop=True)
                    if diag:
                        nc.tensor.matmul(
                            ps[:, t0:t0 + 128], ident_b[:], mask_tri[:],
                            start=False, stop=True, skip_group_check=True)
                    p = p_pool.tile([128, 1024], BF16, name="p_t")
                    with nc.allow_low_precision(reason="softmax p bf16"):
                        nc.scalar.activation(p[:, t0:1024], ps[:, t0:1024],
                                             AF.Exp)
                    with nc.allow_low_precision(reason="den bf16"):
                        if j == 0:
                            nc.vector.tensor_copy(den_acc[h][:], p[:])
                        else:
                            nc.vector.tensor_add(den_acc[h][:, t0:1024],
                                                 den_acc[h][:, t0:1024],
                                                 p[:, t0:1024])
                    p2.append(p)
                if pend is not None:
                    emit_pv(*pend)
                pend = (j, p2, t0)
            emit_pv(*pend)

            # denominator partition-reduce + 1-lane copies
            den_sb = []
            for h in range(2):
                psd = s_pool.tile([128, 1024], FP32, name="ps_s")
                for qs in (0, 512):
                    nc.tensor.matmul(psd[0:1, qs:qs + 512], onescb_sb[:],
                                     den_acc[h][:, qs:qs + 512],
                                     start=True, stop=True,
                                     skip_group_check=True)
                d = dn_pool.tile([1, 1024], FP32, name="dn_t")
                nc.vector.tensor_copy(d[:], psd[0:1, :])
                den_sb.append(d)
            hold = hold_pool.tile([128, 1024], FP32, name="hold_t")
            nc.vector.tensor_copy(hold[:], psum_o[:])
            return hold, den_sb

        def normalize_strip(pools, hold, den_sb, contrib, gathered):
            s_pool, abp, rpp = pools
            recs = []
            for h in range(2):
                rc = rpp.tile([1, 1024], BF16, name="rec_t")
                with nc.allow_low_precision(reason="recip bf16"):
                    nc.vector.reciprocal(rc[:], den_sb[h][:])
                recs.append(rc)
            psw = s_pool.tile([128, 1024], FP32, name="ps_s")
            for h in range(2):
                for qs in (0, 512):
                    nc.tensor.matmul(
                        psw[64 * h:64 * h + 64, qs:qs + 512],
                        ones_bf[0:1, 0:64], recs[h][:, qs:qs + 512],
                        start=True, stop=True, skip_group_check=True)
            ab = abp.tile([128, 1024], BF16, name="ab_t")
            with nc.allow_low_precision(reason="attnorm bf16"):
                nc.vector.tensor_mul(ab[:], hold[:], psw[:])
            nc.sync.dma_start(out=contrib[:], in_=ab[:])
            nc.gpsimd.collective_compute(
                "AllGather", ALU.bypass, replica_groups=GROUPS,
                ins=[contrib.opt()], outs=[gathered.opt()])

        # =============================================== attention + compress
        with ExitStack() as pc:
            s_pool = pc.enter_context(
                tc.tile_pool(name="s_pool", bufs=3, space="PSUM"))
            o_pool = pc.enter_context(
                tc.tile_pool(name="o_pool", bufs=1, space="PSUM"))
            p_pool = pc.enter_context(tc.tile_pool(name="p_pool", bufs=4))
            hold_pool = pc.enter_context(tc.tile_pool(name="hold", bufs=2))
            den_pool = pc.enter_context(tc.tile_pool(name="den", bufs=4))
            dn_pool = pc.enter_context(tc.tile_pool(name="dn", bufs=4))
            abp = pc.enter_context(tc.tile_pool(name="abp", bufs=2))
            att_pools = (s_pool, o_pool, p_pool, hold_pool, den_pool, dn_pool)
            norm_pools = (s_pool, abp, pc.enter_context(
                tc.tile_pool(name="rpp", bufs=2)))

            # prefetch high x chunks (4..7) as halves; they stream during
            # strips 0-1 and feed compress quarters 2-3
            x_hi = {}
            for cc in range(4, 8):
                h0 = xs_pool.tile([128, 2048], BF16, name="xs_t")
                nc.sync.dma_start(out=h0[:], in_=x2t[cc][:, 0:2048])
                h1 = xs_pool.tile([128, 2048], BF16, name="xs_t")
                nc.sync.dma_start(out=h1[:], in_=x2t[cc][:, 2048:4096])
                x_hi[cc] = (h0, h1)

            def compress_quarters(quarters):
                # one quarter: chunk-pair (2cq, 2cq+1) x 4 phases, both halves
                for cq in quarters:
                    psw = s_pool.tile([128, 1024], FP32, name="ps_s")
                    n = 0
                    for cc in (2 * cq, 2 * cq + 1):
                        if cc < 4:
                            srcs = [(x_lo[cc], 0), (x_lo[cc], 2048)]
                        else:
                            srcs = [(x_hi[cc][0], 0), (x_hi[cc][1], 0)]
                        for r in range(4):
                            w = wcp.tile([128, 128], BF16, name="wc_t")
                            nc.sync.dma_start(out=w[:], in_=wc[8 * r + cc])
                            for hf in range(2):
                                xh, off = srcs[hf]
                                nc.tensor.matmul(
                                    psw[:, 512 * hf:512 * hf + 512], w[:],
                                    xh[:, off + r:off + 2048:4],
                                    start=(n == 0), stop=(n == 7))
                            n += 1
                    if cq == 0:
                        nc.vector.tensor_copy(cgT32[:], psw[:])
                    else:
                        nc.vector.tensor_add(cgT32[:], cgT32[:], psw[:])

            holds_l, dens_l = [], []
            for s in range(4):
                h, d = attention_strip(att_pools, 8 * s + 8, 1024 * s,
                                       qT_l, kT_l, v_sb_l)
                normalize_strip(norm_pools, h, d, attl_contrib[s],
                                attl_gathered[s])
                if s == 0:
                    compress_quarters([0, 1])
                elif s == 1:
                    compress_quarters([2, 3])
                    # finalize cg: bias + bf16 cast, AllGather
                    cg_bf = abp.tile([128, 1024], BF16, name="ab_t")
                    with nc.allow_low_precision(reason="cg bf16"):
                        nc.scalar.activation(cg_bf[:], cgT32[:], AF.Identity,
                                             bias=biases["bc"][:])
                    nc.sync.dma_start(out=cg_contrib[:], in_=cg_bf[:])
                    nc.gpsimd.collective_compute(
                        "AllGather", ALU.bypass, replica_groups=GROUPS,
                        ins=[cg_contrib.opt()], outs=[cg_gathered.opt()])
                    for i in range(4):
                        nc.sync.dma_start(
                            out=cg_all[i][:],
                            in_=cg_gathered[128 * i:128 * i + 128, :])
                elif s == 2:
                    # global qkv + v from gathered cg
                    with ExitStack() as pg:
                        wgp = pg.enter_context(
                            tc.tile_pool(name="wg_pool", bufs=1))
                        wgqkA_sb, wgqkB_sb, wgv_sb = [], [], []
                        for cc in range(4):
                            t = wgp.tile([128, 128], BF16, name=f"wgqkA{cc}")
                            nc.sync.dma_start(out=t[:], in_=wgqkA[cc])
                            wgqkA_sb.append(t)
                            t = wgp.tile([128, 128], BF16, name=f"wgqkB{cc}")
                            nc.sync.dma_start(out=t[:], in_=wgqkB[cc])
                            wgqkB_sb.append(t)
                            t = wgp.tile([128, 128], BF16, name=f"wgv{cc}")
                            nc.sync.dma_start(out=t[:], in_=wgv[cc])
                            wgv_sb.append(t)
                        for (wts, bias_ap, dst) in (
                                (wgqkA_sb, biases["bgqkA"][:], qT_g),
                                (wgqkB_sb, biases["bgqkB"][:], kT_g)):
                            for qt in range(2):
                                psw = s_pool.tile([128, 1024], FP32,
                                                  name="ps_s")
                                ps = psw[:, 0:512]
                                for cc in range(4):
                                    nc.tensor.matmul(
                                        ps, wts[cc][:],
                                        cg_all[cc][:, 512 * qt:512 * qt + 512],
                                        start=(cc == 0), stop=(cc == 3))
                                with nc.allow_low_precision(reason="gqk bf16"):
                                    nc.scalar.activation(
                                        dst[:, 512 * qt:512 * qt + 512],
                                        ps, AF.Identity, bias=bias_ap)
                        for kb in range(8):
                            psw = s_pool.tile([128, 1024], FP32, name="ps_s")
                            ps = psw[:, 0:128]
                            for cc in range(4):
                                nc.tensor.matmul(
                                    ps, cg_all[cc][:, 128 * kb:128 * kb + 128],
                                    wgv_sb[cc][:], start=(cc == 0), stop=False)
                            nc.tensor.matmul(ps, ones_bf[:, 0:128],
                                             bgvr_sb[:],
                                             start=False, stop=True)
                            with nc.allow_low_precision(reason="gv bf16"):
                                nc.vector.tensor_copy(v_sb_g[kb][:], ps)

            # ------------------------------------------- global attention
            gh, gd = attention_strip(att_pools, 8, 0, qT_g, kT_g, v_sb_g)
            normalize_strip(norm_pools, gh, gd, attg_contrib, attg_gathered)

        pab.close()

        # =============================================== proj + gate + out
        with ExitStack() as pd:
            ap_pool = pd.enter_context(tc.tile_pool(name="attall", bufs=1))
            wpp = pd.enter_context(tc.tile_pool(name="wp_pool", bufs=1))
            psP = pd.enter_context(tc.tile_pool(name="psP", bufs=2,
                                                space="PSUM"))
            psE = pd.enter_context(tc.tile_pool(name="psE", bufs=2,
                                                space="PSUM"))
            psD = pd.enter_context(tc.tile_pool(name="psD", bufs=1,
                                                space="PSUM"))
            gp = pd.enter_context(tc.tile_pool(name="g_pool", bufs=1))
            outp = pd.enter_context(tc.tile_pool(name="out_pool", bufs=4))
            stp = pd.enter_context(tc.tile_pool(name="stage_pool", bufs=1))

            att_all, attg_all = [], []
            for ch in range(4):
                t = ap_pool.tile([128, T], BF16, name=f"attall{ch}")
                for s in range(4):
                    nc.sync.dma_start(
                        out=t[:, 1024 * s:1024 * s + 1024],
                        in_=attl_gathered[s][128 * ch:128 * ch + 128, :])
                att_all.append(t)
                t2 = ap_pool.tile([128, Tc], BF16, name=f"attgall{ch}")
                nc.sync.dma_start(out=t2[:],
                                  in_=attg_gathered[128 * ch:128 * ch + 128, :])
                attg_all.append(t2)
            wpl_sb, wpg_sb = [], []
            for ch in range(4):
                t = wpp.tile([128, 129], BF16, name=f"wpl{ch}")
                nc.sync.dma_start(out=t[:], in_=wpl[ch])
                wpl_sb.append(t)
                t = wpp.tile([128, 129], BF16, name=f"wpg{ch}")
                nc.sync.dma_start(out=t[:], in_=wpg[ch])
                wpg_sb.append(t)

            # global proj first (its gate column feeds the local gate)
            dg_sb = gp.tile([128, 8], BF16, name="dg_sb")
            gstage = []
            for tbg in range(8):
                ps = psP.tile([128, 129], FP32, name="psPg_t")
                for ch in range(4):
                    nc.tensor.matmul(
                        ps[:], attg_all[ch][:, 128 * tbg:128 * tbg + 128],
                        wpg_sb[ch][:], start=(ch == 0), stop=False)
                nc.tensor.matmul(ps[:], ones_bf[:, 0:128], bpge_sb[:],
                                 start=False, stop=True)
                with nc.allow_low_precision(reason="gate logit bf16"):
                    nc.vector.tensor_copy(dg_sb[:, tbg:tbg + 1],
                                          ps[:, 128:129])
                gt = gp.tile([128, 128], BF16, name=f"gst{tbg}")
                with nc.allow_low_precision(reason="gproj bf16 for expand"):
                    nc.vector.tensor_copy(gt[:], ps[:, 0:128])
                gstage.append(gt)

            # expand dg x4 into natural token blocks: dgx [128, 32]
            ps_dgx = psD.tile([128, 32], FP32, name="ps_dgx")
            for tb in range(32):
                base = 64 * ((tb % 4) // 2)
                rep = repA_sb if tb % 2 == 0 else repB_sb
                nc.tensor.matmul(ps_dgx[:, tb:tb + 1],
                                 rep[base:base + 64, :],
                                 dg_sb[base:base + 64, tb // 4:tb // 4 + 1],
                                 start=True, stop=True, skip_group_check=True)

            # local proj: psum -> outstage + dl column
            dl_sb = gp.tile([128, 32], FP32, name="dl_sb")
            outst = []
            for tb in range(32):
                ps = psP.tile([128, 129], FP32, name="psPl_t")
                for ch in range(4):
                    nc.tensor.matmul(
                        ps[:], att_all[ch][:, 128 * tb:128 * tb + 128],
                        wpl_sb[ch][:], start=(ch == 0), stop=False)
                nc.tensor.matmul(ps[:], ones_bf[:, 0:128], bple_sb[:],
                                 start=False, stop=True)
                nc.vector.tensor_copy(dl_sb[:, tb:tb + 1], ps[:, 128:129])
                ot = stp.tile([128, 128], FP32, name=f"outst{tb}")
                nc.scalar.activation(ot[:], ps[:, 0:128], AF.Copy)
                outst.append(ot)

            # gate: tanh(0.5*(dl+dgx) + 0.5*c0)
            dsum = gp.tile([128, 32], FP32, name="dsum")
            nc.vector.tensor_add(dsum[:], dl_sb[:], ps_dgx[:])
            tanh_sb = gp.tile([128, 32], FP32, name="tanh_sb")
            nc.scalar.activation(tanh_sb[:], dsum[:], AF.Tanh,
                                 scale=0.5, bias=biases["c0h"][:])
            g0 = gp.tile([128, 32], FP32, name="g0")
            g1 = gp.tile([128, 32], FP32, name="g1")
            nc.vector.tensor_scalar(g0[:], tanh_sb[:], 0.5, 0.5,
                                    ALU.mult, ALU.add)
            nc.vector.tensor_scalar(g1[:], tanh_sb[:], -0.5, 0.5,
                                    ALU.mult, ALU.add)

            for tb in range(32):
                o = outp.tile([128, 128], FP32, name="outl")
                nc.vector.tensor_scalar_mul(o[:], outst[tb][:],
                                            g0[:, tb:tb + 1])
                nc.sync.dma_start(out=out_loc[128 * tb:128 * tb + 128, :],
                                  in_=o[:])
            for tb in range(32):
                ps = psE.tile([128, 128], FP32, name="psE_t")
                base = 64 * ((tb % 4) // 2)
                rep = repA_sb if tb % 2 == 0 else repB_sb
                nc.tensor.matmul(ps[:], rep[base:base + 64, :],
                                 gstage[tb // 4][base:base + 64, :],
                                 start=True, stop=True)
                o = outp.tile([128, 128], FP32, name="outg")
                nc.vector.tensor_scalar_mul(o[:], ps[:], g1[:, tb:tb + 1])
                nc.sync.dma_start(out=out_glob[128 * tb:128 * tb + 128, :],
                                  in_=o[:])

    nc.finalize()
    return nc


# ---------------------------------------------------------------------------
# Host side
# ---------------------------------------------------------------------------

_NC_CACHE = []


def _get_program():
    if not _NC_CACHE:
        _NC_CACHE.append(build_program())
    return _NC_CACHE[0]


def _prep_inputs(x, w_lqkv, b_lqkv, w_gqkv, b_gqkv, w_comp, b_comp,
                 w_lproj, b_lproj, w_gproj, b_gproj, w_gate, b_gate):
    import ml_dtypes
    f32 = np.float32
    bf16 = ml_dtypes.bfloat16
    wd = (w_gate[:, 0] - w_gate[:, 1]).astype(f32)
    u_l = (w_lproj @ wd[:LD]).astype(f32)
    u_g = (w_gproj @ wd[LD:]).astype(f32)
    c0 = float(b_lproj @ wd[:LD] + b_gproj @ wd[LD:] + b_gate[0] - b_gate[1])

    mask_tri = np.where(np.arange(128)[None, :] >= np.arange(128)[:, None],
                        0.0, NEG).astype(f32)
    e0 = np.zeros((64, 128), f32)
    e0[np.arange(128) // 4, np.arange(128)] = 1.0
    e1 = np.zeros((64, 128), f32)
    e1[32 + np.arange(128) // 4, np.arange(128)] = 1.0
    repA_ = np.concatenate([e0, e0], axis=0)
    repB_ = np.concatenate([e1, e1], axis=0)
    sel2_ = np.zeros((2, 128), f32)
    sel2_[0, 0:64] = 1.0
    sel2_[1, 64:128] = 1.0

    def qk_packed(wqkv, bqkv, ha, hb):
        wA = np.concatenate([wqkv[:, D * ha:D * ha + D] / 8.0,
                             wqkv[:, D * hb:D * hb + D] / 8.0], axis=1)
        bA = np.concatenate([bqkv[D * ha:D * ha + D] / 8.0,
                             bqkv[D * hb:D * hb + D] / 8.0])
        wB = np.concatenate([wqkv[:, LD + D * ha:LD + D * ha + D],
                             wqkv[:, LD + D * hb:LD + D * hb + D]], axis=1)
        bB = np.concatenate([bqkv[LD + D * ha:LD + D * ha + D],
                             bqkv[LD + D * hb:LD + D * hb + D]])
        return (wA.reshape(4, 128, 128), bA.astype(f32).reshape(128, 1),
                wB.reshape(4, 128, 128), bB.astype(f32).reshape(128, 1))

    def v_packed(wqkv, bqkv, ha, hb):
        wv_ = np.concatenate([wqkv[:, 2 * LD + D * ha:2 * LD + D * ha + D],
                              wqkv[:, 2 * LD + D * hb:2 * LD + D * hb + D]],
                             axis=1)
        bv_ = np.concatenate([bqkv[2 * LD + D * ha:2 * LD + D * ha + D],
                              bqkv[2 * LD + D * hb:2 * LD + D * hb + D]])
        return wv_.reshape(4, 128, 128), bv_.astype(f32).reshape(1, 128)

    in_maps = []
    for core in range(NCORES):
        b_idx, g = core // 4, core % 4
        ha, hb = 2 * g, 2 * g + 1
        cs = slice(128 * g, 128 * g + 128)

        x2t_ = np.ascontiguousarray(x[b_idx].T).reshape(8, 128, T)
        wc_s = np.ascontiguousarray(
            w_comp[:, LD + 128 * g:LD + 128 * g + 128]).reshape(32, 128, 128)
        bc_s = b_comp[LD + 128 * g:LD + 128 * g + 128].astype(f32) \
            .reshape(128, 1)

        wqkA_, bqkA_, wqkB_, bqkB_ = qk_packed(w_lqkv, b_lqkv, ha, hb)
        wv_, bvr_ = v_packed(w_lqkv, b_lqkv, ha, hb)
        wgqkA_, bgqkA_, wgqkB_, bgqkB_ = qk_packed(w_gqkv, b_gqkv, ha, hb)
        wgv_, bgvr_ = v_packed(w_gqkv, b_gqkv, ha, hb)

        wpl_ = np.concatenate([w_lproj[:, cs], u_l[:, None]],
                              axis=1).reshape(4, 128, 129)
        wpg_ = np.concatenate([w_gproj[:, cs], u_g[:, None]],
                              axis=1).reshape(4, 128, 129)
        bple_ = np.concatenate([b_lproj[cs], [0.0]]).astype(f32) \
            .reshape(1, 129)
        bpge_ = np.concatenate([b_gproj[cs], [0.0]]).astype(f32) \
            .reshape(1, 129)

        m = {
            "x2t": x2t_, "wc": wc_s, "bc": bc_s,
            "wqkA": wqkA_, "bqkA": bqkA_, "wqkB": wqkB_, "bqkB": bqkB_,
            "wv": wv_, "bvr": bvr_,
            "wgqkA": wgqkA_, "bgqkA": bgqkA_, "wgqkB": wgqkB_,
            "bgqkB": bgqkB_, "wgv": wgv_, "bgvr": bgvr_,
            "wpl": wpl_, "wpg": wpg_, "bple": bple_, "bpge": bpge_,
            "c0h": np.full((128, 1), 0.5 * c0, f32),
            "onesr": np.ones((1, 129), f32),
            "onescb": np.ones((128, 1), f32),
            "repA": repA_, "repB": repB_, "maskt": mask_tri,
        }
        for k in ("x2t", "wc", "wqkA", "wqkB", "wv", "bvr", "wgqkA", "wgqkB",
                  "wgv", "bgvr", "wpl", "wpg", "bple", "bpge", "onesr",
                  "onescb", "repA", "repB", "maskt"):
            m[k] = m[k].astype(bf16)
        in_maps.append(m)
    return in_maps


def _run(in_maps, trace=False):
    nc = _get_program()
    return run_bass_kernel_spmd(nc, in_maps, list(range(NCORES)), trace=trace)


def assemble(results):
    out = np.empty((B, T, E), np.float32)
    for core in range(NCORES):
        b_idx, g = core // 4, core % 4
        out[b_idx, :, 128 * g:128 * g + 128] = results[core]["out_loc"]
        out[b_idx, :, LD + 128 * g:LD + 128 * g + 128] = \
            results[core]["out_glob"]
    return out


def kernel(**inputs):
    in_maps = _prep_inputs(**inputs)
    res = _run(in_maps)
    return assemble(res.results)


def kernel_traced(**inputs):
    in_maps = _prep_inputs(**inputs)
    res = _run(in_maps, trace=True)
    return assemble(res.results), res



# revision 30
# speedup vs baseline: 1.1522x; 1.1522x over previous
"""DualResolutionAttention Trainium2 kernel v3 (8 NeuronCores, Bass/Tile).

Sharding: core c -> (batch b = c//4, group g = c%4); heads {2g, 2g+1} both
branches; output channel slice [128g, 128g+128) of each branch.

v3 vs v2:
- whole datapath bf16 (x input shipped bf16): LDWEIGHTS pipelines (no fp32r
  self-load tax), input DMA halves to 8MB
- attention: 3-way rotating [128,1024] score psum tiles + per-head exp ->
  scalar engine stays saturated (no scores->exp serialization)
- PV col-tiled packed: h0 -> psum partitions 0:64, h1 -> 64:128, one
  [128,1024] accumulator (2 banks)
- softmax denominators: DVE-accumulated den_acc (bf16) + ones-column matmul
  partition-reduce; no ones-column in v, no stag reshape machinery
- compress split into 4 quarter-chunks run at strip boundaries, accumulated
  in SBUF via DVE adds; cg AllGather overlaps strips 2-3
"""
import os
import sys

sys.path.insert(0, "/opt/trn_rl_repo")
os.environ.setdefault("JAX_PLATFORMS", "axon,cpu")

from contextlib import ExitStack

import numpy as np

import concourse.bass as bass
import concourse.mybir as mybir
import concourse.tile as tile
from concourse import bacc
from concourse.bass_utils import run_bass_kernel_spmd
from concourse.masks import make_identity

FP32 = mybir.dt.float32
FP32R = mybir.dt.float32r
BF16 = mybir.dt.bfloat16
AF = mybir.ActivationFunctionType
ALU = mybir.AluOpType

B, T, E = 2, 4096, 1024
LD = 512
D = 64
R = 4
Tc = T // R
NCORES = 8
GROUPS = [[0, 1, 2, 3], [4, 5, 6, 7]]
NEG = -1.0e9


def build_program():
    nc = bacc.Bacc(None, target_bir_lowering=False)

    def inp(name, shape, dt=BF16):
        return nc.declare_dram_parameter(name, list(shape), dt, isOutput=False)

    x2t = inp("x2t", [8, 128, T])            # x[b].T chunks, bf16
    wc = inp("wc", [32, 128, 128])           # compress slice lhsT chunks
    bc = inp("bc", [128, 1], FP32)
    wqkA = inp("wqkA", [4, 128, 128])        # [q_h0/8 | q_h1/8] lhsT chunks
    wqkB = inp("wqkB", [4, 128, 128])        # [k_h0 | k_h1] lhsT chunks
    bqkA = inp("bqkA", [128, 1], FP32)
    bqkB = inp("bqkB", [128, 1], FP32)
    wv = inp("wv", [4, 128, 128])            # [v_h0 | v_h1] rhs chunks
    bvr = inp("bvr", [1, 128])               # [bv_h0 | bv_h1] bias row
    wgqkA = inp("wgqkA", [4, 128, 128])
    wgqkB = inp("wgqkB", [4, 128, 128])
    bgqkA = inp("bgqkA", [128, 1], FP32)
    bgqkB = inp("bgqkB", [128, 1], FP32)
    wgv = inp("wgv", [4, 128, 128])
    bgvr = inp("bgvr", [1, 128])
    wpl = inp("wpl", [4, 128, 129])          # [w_lproj slice | u_l] chunks
    wpg = inp("wpg", [4, 128, 129])
    bple = inp("bple", [1, 129])             # [b_lproj slice, 0]
    bpge = inp("bpge", [1, 129])
    c0h = inp("c0h", [128, 1], FP32)         # 0.5 * gate const
    onesr = inp("onesr", [1, 129])           # ones row (bf16)
    onescb = inp("onescb", [128, 1])         # ones column (bf16)
    repA = inp("repA", [128, 128])           # x4 expander (even 32-blocks)
    repB = inp("repB", [128, 128])
    maskt = inp("maskt", [128, 128])         # strict lower-tri NEG
    out_loc = nc.declare_dram_parameter("out_loc", [T, 128], FP32, isOutput=True)
    out_glob = nc.declare_dram_parameter("out_glob", [T, 128], FP32,
                                         isOutput=True)

    with tile.TileContext(nc) as tc:
      with ExitStack() as top:
        dram = top.enter_context(tc.tile_pool(name="dram", bufs=1, space="DRAM"))
        const = top.enter_context(tc.tile_pool(name="const", bufs=1))
        persist = top.enter_context(tc.tile_pool(name="persist", bufs=1))

        # ---- constants
        ident_b = const.tile([128, 128], BF16, name="ident_b")
        make_identity(nc, ident_b[:])
        mask_tri = const.tile([128, 128], BF16, name="mask_tri")
        nc.sync.dma_start(out=mask_tri[:], in_=maskt[:])
        repA_sb = const.tile([128, 128], BF16, name="repA_sb")
        nc.sync.dma_start(out=repA_sb[:], in_=repA[:])
        repB_sb = const.tile([128, 128], BF16, name="repB_sb")
        nc.sync.dma_start(out=repB_sb[:], in_=repB[:])
        onescb_sb = const.tile([128, 1], BF16, name="onescb_sb")
        nc.sync.dma_start(out=onescb_sb[:], in_=onescb[:])
        ones_bf = const.tile([1, 129], BF16, name="ones_bf")
        nc.sync.dma_start(out=ones_bf[:], in_=onesr[:])
        bple_sb = const.tile([1, 129], BF16, name="bple_sb")
        nc.sync.dma_start(out=bple_sb[:], in_=bple[:])
        bpge_sb = const.tile([1, 129], BF16, name="bpge_sb")
        nc.sync.dma_start(out=bpge_sb[:], in_=bpge[:])
        bvr_sb = const.tile([1, 128], BF16, name="bvr_sb")
        nc.sync.dma_start(out=bvr_sb[:], in_=bvr[:])
        bgvr_sb = const.tile([1, 128], BF16, name="bgvr_sb")
        nc.sync.dma_start(out=bgvr_sb[:], in_=bgvr[:])
        biases = {}
        for nm, src in (("bc", bc), ("c0h", c0h),
                        ("bqkA", bqkA), ("bqkB", bqkB),
                        ("bgqkA", bgqkA), ("bgqkB", bgqkB)):
            t = const.tile([128, 1], FP32, name=f"cb_{nm}")
            nc.sync.dma_start(out=t[:], in_=src[:])
            biases[nm] = t

        # ---- persistent tensors (all bf16)
        qT_l = persist.tile([128, T], BF16, name="qT_l")
        kT_l = persist.tile([128, T], BF16, name="kT_l")
        qT_g = persist.tile([128, Tc], BF16, name="qT_g")
        kT_g = persist.tile([128, Tc], BF16, name="kT_g")
        v_sb_l = [persist.tile([128, 128], BF16, name=f"vsb{i}")
                  for i in range(32)]
        v_sb_g = [persist.tile([128, 128], BF16, name=f"vgsb{i}")
                  for i in range(8)]
        cg_all = [persist.tile([128, Tc], BF16, name=f"cg_all{i}")
                  for i in range(4)]
        cgT32 = persist.tile([128, Tc], FP32, name="cgT32")

        # DRAM bounce tiles for collectives
        cg_contrib = dram.tile([128, Tc], BF16, name="cg_contrib")
        cg_gathered = dram.tile([512, Tc], BF16, name="cg_gathered")
        attl_contrib = [dram.tile([128, 1024], BF16, name=f"alc{s}")
                        for s in range(4)]
        attl_gathered = [dram.tile([512, 1024], BF16, name=f"alg{s}")
                         for s in range(4)]
        attg_contrib = dram.tile([128, Tc], BF16, name="agc")
        attg_gathered = dram.tile([512, Tc], BF16, name="agg")

        # =========================================== phase AB: x load, qkv, v
        pab = top.enter_context(ExitStack())
        xq_pool = pab.enter_context(tc.tile_pool(name="xq_pool", bufs=1))
        xs_pool = pab.enter_context(tc.tile_pool(name="xs_pool", bufs=8))
        w_pool = pab.enter_context(tc.tile_pool(name="w_pool", bufs=1))
        wcp = pab.enter_context(tc.tile_pool(name="wc_pool", bufs=8))

        x_lo = []
        for cc in range(4):
            t = xq_pool.tile([128, T], BF16, name=f"x2t{cc}")
            nc.sync.dma_start(out=t[:], in_=x2t[cc])
            x_lo.append(t)

        wqkA_sb, wqkB_sb, wv_sb = [], [], []
        for cc in range(4):
            t = w_pool.tile([128, 128], BF16, name=f"wqkA{cc}")
            nc.sync.dma_start(out=t[:], in_=wqkA[cc])
            wqkA_sb.append(t)
            t = w_pool.tile([128, 128], BF16, name=f"wqkB{cc}")
            nc.sync.dma_start(out=t[:], in_=wqkB[cc])
            wqkB_sb.append(t)
            t = w_pool.tile([128, 128], BF16, name=f"wv{cc}")
            nc.sync.dma_start(out=t[:], in_=wv[cc])
            wv_sb.append(t)

        with ExitStack() as pqkv:
            psAB = pqkv.enter_context(
                tc.tile_pool(name="psAB", bufs=2, space="PSUM"))
            psV = pqkv.enter_context(
                tc.tile_pool(name="psV", bufs=2, space="PSUM"))
            # local q/k: packed tiles -> direct DVE copy
            for (wts, bias_ap, dst) in ((wqkA_sb, biases["bqkA"][:], qT_l),
                                        (wqkB_sb, biases["bqkB"][:], kT_l)):
                for qt in range(8):
                    ps = psAB.tile([128, 512], FP32, name="psAB_t")
                    for cc in range(4):
                        nc.tensor.matmul(
                            ps[:], wts[cc][:],
                            x_lo[cc][:, 512 * qt:512 * qt + 512],
                            start=(cc == 0), stop=(cc == 3))
                    with nc.allow_low_precision(reason="qk bf16"):
                        nc.scalar.activation(dst[:, 512 * qt:512 * qt + 512],
                                             ps[:], AF.Identity, bias=bias_ap)
            # local v: x-stationary, v_sb[t, d] direct
            for kb in range(32):
                ps = psV.tile([128, 128], FP32, name="psV_t")
                for cc in range(4):
                    nc.tensor.matmul(
                        ps[:], x_lo[cc][:, 128 * kb:128 * kb + 128],
                        wv_sb[cc][:], start=(cc == 0), stop=False)
                nc.tensor.matmul(ps[:], ones_bf[:, 0:128], bvr_sb[:],
                                 start=False, stop=True)
                with nc.allow_low_precision(reason="v bf16"):
                    nc.vector.tensor_copy(v_sb_l[kb][:], ps[:])

        # =============================================== attention machinery
        def attention_strip(pools, nkb, q0, qT, kT, v_sb):
            """One 1024-q strip, both heads; returns (hold, den_sb[2])."""
            s_pool, o_pool, p_pool, hold_pool, den_pool, dn_pool = pools
            psum_o = o_pool.tile([128, 1024], FP32, name="ps_o")
            den_acc = [den_pool.tile([128, 1024], BF16, name="den_t")
                       for _ in range(2)]
            pend = None

            def emit_pv(j, ps_list, t0):
                for h in range(2):
                    for qs in (0, 512):
                        lo = max(qs, t0)
                        hi = qs + 512
                        if lo >= hi:
                            continue
                        nc.tensor.matmul(
                            psum_o[64 * h:64 * h + 64, lo:hi],
                            v_sb[j][:, 64 * h:64 * h + 64],
                            ps_list[h][:, lo:hi],
                            start=(j == 0), stop=(j == nkb - 1),
                            skip_group_check=True)

            for j in range(nkb):
                t0 = max(0, 128 * j - q0)
                diag = 128 * j >= q0
                p2 = []
                for h in range(2):
                    ps = s_pool.tile([128, 1024], FP32, name="ps_s")
                    for qs in (0, 512):
                        if qs + 512 <= (t0 // 512) * 512:
                            continue
                        nc.tensor.matmul(
                            ps[:, qs:qs + 512],
                            kT[64 * h:64 * h + 64, 128 * j:128 * j + 128],
                            qT[64 * h:64 * h + 64, q0 + qs:q0 + qs + 512],
                            start=True, stop=True)
                    if diag:
                        nc.tensor.matmul(
                            ps[:, t0:t0 + 128], ident_b[:], mask_tri[:],
                            start=False, stop=True, skip_group_check=True)
                    p = p_pool.tile([128, 1024], BF16, name="p_t")
                    with nc.allow_low_precision(reason="softmax p bf16"):
                        nc.scalar.activation(p[:, t0:1024], ps[:, t0:1024],
                                             AF.Exp)
                    with nc.allow_low_precision(reason="den bf16"):
                        if j == 0:
                            nc.vector.tensor_copy(den_acc[h][:], p[:])
                        else:
                            nc.vector.tensor_add(den_acc[h][:, t0:1024],
                                                 den_acc[h][:, t0:1024],
                                                 p[:, t0:1024])
                    p2.append(p)
                if pend is not None:
                    emit_pv(*pend)
                pend = (j, p2, t0)
            emit_pv(*pend)

            # denominator partition-reduce + 1-lane copies
            den_sb = []
            for h in range(2):
                psd = s_pool.tile([128, 1024], FP32, name="ps_s")
                for qs in (0, 512):
                    nc.tensor.matmul(psd[0:1, qs:qs + 512], onescb_sb[:],
                                     den_acc[h][:, qs:qs + 512],
                                     start=True, stop=True,
                                     skip_group_check=True)
                d = dn_pool.tile([1, 1024], FP32, name="dn_t")
                nc.vector.tensor_copy(d[:], psd[0:1, :])
                den_sb.append(d)
            hold = hold_pool.tile([128, 1024], FP32, name="hold_t")
            nc.vector.tensor_copy(hold[:], psum_o[:])
            return hold, den_sb

        def normalize_strip(pools, hold, den_sb, contrib, gathered):
            s_pool, abp, rpp = pools
            recs = []
            for h in range(2):
                rc32 = rpp.tile([1, 1024], FP32, name="rec32_t")
                nc.vector.reciprocal_approx_fast(out=rc32[:],
                                                 in_=den_sb[h][:])
                rc = rpp.tile([1, 1024], BF16, name="rec_t")
                with nc.allow_low_precision(reason="recip bf16"):
                    nc.vector.tensor_copy(rc[:], rc32[:])
                recs.append(rc)
            psw = s_pool.tile([128, 1024], FP32, name="ps_s")
            for h in range(2):
                for qs in (0, 512):
                    nc.tensor.matmul(
                        psw[64 * h:64 * h + 64, qs:qs + 512],
                        ones_bf[0:1, 0:64], recs[h][:, qs:qs + 512],
                        start=True, stop=True, skip_group_check=True)
            ab = abp.tile([128, 1024], BF16, name="ab_t")
            with nc.allow_low_precision(reason="attnorm bf16"):
                nc.vector.tensor_mul(ab[:], hold[:], psw[:])
            nc.sync.dma_start(out=contrib[:], in_=ab[:])
            nc.gpsimd.collective_compute(
                "AllGather", ALU.bypass, replica_groups=GROUPS,
                ins=[contrib.opt()], outs=[gathered.opt()])

        # =============================================== attention + compress
        with ExitStack() as pc:
            s_pool = pc.enter_context(
                tc.tile_pool(name="s_pool", bufs=3, space="PSUM"))
            o_pool = pc.enter_context(
                tc.tile_pool(name="o_pool", bufs=1, space="PSUM"))
            p_pool = pc.enter_context(tc.tile_pool(name="p_pool", bufs=4))
            hold_pool = pc.enter_context(tc.tile_pool(name="hold", bufs=2))
            den_pool = pc.enter_context(tc.tile_pool(name="den", bufs=4))
            dn_pool = pc.enter_context(tc.tile_pool(name="dn", bufs=4))
            abp = pc.enter_context(tc.tile_pool(name="abp", bufs=2))
            att_pools = (s_pool, o_pool, p_pool, hold_pool, den_pool, dn_pool)
            norm_pools = (s_pool, abp, pc.enter_context(
                tc.tile_pool(name="rpp", bufs=2)))

            # prefetch high x chunks (4..7) as halves; they stream during
            # strips 0-1 and feed compress quarters 2-3
            x_hi = {}
            for cc in range(4, 8):
                h0 = xs_pool.tile([128, 2048], BF16, name="xs_t")
                nc.sync.dma_start(out=h0[:], in_=x2t[cc][:, 0:2048])
                h1 = xs_pool.tile([128, 2048], BF16, name="xs_t")
                nc.sync.dma_start(out=h1[:], in_=x2t[cc][:, 2048:4096])
                x_hi[cc] = (h0, h1)

            def compress_quarters(quarters):
                # one quarter: chunk-pair (2cq, 2cq+1) x 4 phases, both halves
                for cq in quarters:
                    psw = s_pool.tile([128, 1024], FP32, name="ps_s")
                    n = 0
                    for cc in (2 * cq, 2 * cq + 1):
                        if cc < 4:
                            srcs = [(x_lo[cc], 0), (x_lo[cc], 2048)]
                        else:
                            srcs = [(x_hi[cc][0], 0), (x_hi[cc][1], 0)]
                        for r in range(4):
                            w = wcp.tile([128, 128], BF16, name="wc_t")
                            nc.sync.dma_start(out=w[:], in_=wc[8 * r + cc])
                            for hf in range(2):
                                xh, off = srcs[hf]
                                nc.tensor.matmul(
                                    psw[:, 512 * hf:512 * hf + 512], w[:],
                                    xh[:, off + r:off + 2048:4],
                                    start=(n == 0), stop=(n == 7))
                            n += 1
                    if cq == 0:
                        nc.vector.tensor_copy(cgT32[:], psw[:])
                    else:
                        nc.vector.tensor_add(cgT32[:], cgT32[:], psw[:])

            holds_l, dens_l = [], []
            for s in range(4):
                h, d = attention_strip(att_pools, 8 * s + 8, 1024 * s,
                                       qT_l, kT_l, v_sb_l)
                normalize_strip(norm_pools, h, d, attl_contrib[s],
                                attl_gathered[s])
                if s == 0:
                    compress_quarters([0, 1])
                elif s == 1:
                    compress_quarters([2, 3])
                    # finalize cg: bias + bf16 cast, AllGather
                    cg_bf = abp.tile([128, 1024], BF16, name="ab_t")
                    with nc.allow_low_precision(reason="cg bf16"):
                        nc.scalar.activation(cg_bf[:], cgT32[:], AF.Identity,
                                             bias=biases["bc"][:])
                    nc.sync.dma_start(out=cg_contrib[:], in_=cg_bf[:])
                    nc.gpsimd.collective_compute(
                        "AllGather", ALU.bypass, replica_groups=GROUPS,
                        ins=[cg_contrib.opt()], outs=[cg_gathered.opt()])
                    for i in range(4):
                        nc.sync.dma_start(
                            out=cg_all[i][:],
                            in_=cg_gathered[128 * i:128 * i + 128, :])
                elif s == 2:
                    # global qkv + v from gathered cg
                    with ExitStack() as pg:
                        wgp = pg.enter_context(
                            tc.tile_pool(name="wg_pool", bufs=1))
                        wgqkA_sb, wgqkB_sb, wgv_sb = [], [], []
                        for cc in range(4):
                            t = wgp.tile([128, 128], BF16, name=f"wgqkA{cc}")
                            nc.sync.dma_start(out=t[:], in_=wgqkA[cc])
                            wgqkA_sb.append(t)
                            t = wgp.tile([128, 128], BF16, name=f"wgqkB{cc}")
                            nc.sync.dma_start(out=t[:], in_=wgqkB[cc])
                            wgqkB_sb.append(t)
                            t = wgp.tile([128, 128], BF16, name=f"wgv{cc}")
                            nc.sync.dma_start(out=t[:], in_=wgv[cc])
                            wgv_sb.append(t)
                        for (wts, bias_ap, dst) in (
                                (wgqkA_sb, biases["bgqkA"][:], qT_g),
                                (wgqkB_sb, biases["bgqkB"][:], kT_g)):
                            for qt in range(2):
                                psw = s_pool.tile([128, 1024], FP32,
                                                  name="ps_s")
                                ps = psw[:, 0:512]
                                for cc in range(4):
                                    nc.tensor.matmul(
                                        ps, wts[cc][:],
                                        cg_all[cc][:, 512 * qt:512 * qt + 512],
                                        start=(cc == 0), stop=(cc == 3))
                                with nc.allow_low_precision(reason="gqk bf16"):
                                    nc.scalar.activation(
                                        dst[:, 512 * qt:512 * qt + 512],
                                        ps, AF.Identity, bias=bias_ap)
                        for kb in range(8):
                            psw = s_pool.tile([128, 1024], FP32, name="ps_s")
                            ps = psw[:, 0:128]
                            for cc in range(4):
                                nc.tensor.matmul(
                                    ps, cg_all[cc][:, 128 * kb:128 * kb + 128],
                                    wgv_sb[cc][:], start=(cc == 0), stop=False)
                            nc.tensor.matmul(ps, ones_bf[:, 0:128],
                                             bgvr_sb[:],
                                             start=False, stop=True)
                            with nc.allow_low_precision(reason="gv bf16"):
                                nc.vector.tensor_copy(v_sb_g[kb][:], ps)

            # ------------------------------------------- global attention
            gh, gd = attention_strip(att_pools, 8, 0, qT_g, kT_g, v_sb_g)
            normalize_strip(norm_pools, gh, gd, attg_contrib, attg_gathered)

        pab.close()

        # =============================================== proj + gate + out
        with ExitStack() as pd:
            ap_pool = pd.enter_context(tc.tile_pool(name="attall", bufs=1))
            wpp = pd.enter_context(tc.tile_pool(name="wp_pool", bufs=1))
            psP = pd.enter_context(tc.tile_pool(name="psP", bufs=2,
                                                space="PSUM"))
            psE = pd.enter_context(tc.tile_pool(name="psE", bufs=2,
                                                space="PSUM"))
            psD = pd.enter_context(tc.tile_pool(name="psD", bufs=1,
                                                space="PSUM"))
            gp = pd.enter_context(tc.tile_pool(name="g_pool", bufs=1))
            outp = pd.enter_context(tc.tile_pool(name="out_pool", bufs=4))
            stp = pd.enter_context(tc.tile_pool(name="stage_pool", bufs=1))

            att_all, attg_all = [], []
            for ch in range(4):
                t = ap_pool.tile([128, T], BF16, name=f"attall{ch}")
                for s in range(4):
                    nc.sync.dma_start(
                        out=t[:, 1024 * s:1024 * s + 1024],
                        in_=attl_gathered[s][128 * ch:128 * ch + 128, :])
                att_all.append(t)
                t2 = ap_pool.tile([128, Tc], BF16, name=f"attgall{ch}")
                nc.sync.dma_start(out=t2[:],
                                  in_=attg_gathered[128 * ch:128 * ch + 128, :])
                attg_all.append(t2)
            wpl_sb, wpg_sb = [], []
            for ch in range(4):
                t = wpp.tile([128, 129], BF16, name=f"wpl{ch}")
                nc.sync.dma_start(out=t[:], in_=wpl[ch])
                wpl_sb.append(t)
                t = wpp.tile([128, 129], BF16, name=f"wpg{ch}")
                nc.sync.dma_start(out=t[:], in_=wpg[ch])
                wpg_sb.append(t)

            # global proj first (its gate column feeds the local gate)
            dg_sb = gp.tile([128, 8], BF16, name="dg_sb")
            gstage = []
            for tbg in range(8):
                ps = psP.tile([128, 129], FP32, name="psPg_t")
                for ch in range(4):
                    nc.tensor.matmul(
                        ps[:], attg_all[ch][:, 128 * tbg:128 * tbg + 128],
                        wpg_sb[ch][:], start=(ch == 0), stop=False)
                nc.tensor.matmul(ps[:], ones_bf[:, 0:128], bpge_sb[:],
                                 start=False, stop=True)
                with nc.allow_low_precision(reason="gate logit bf16"):
                    nc.vector.tensor_copy(dg_sb[:, tbg:tbg + 1],
                                          ps[:, 128:129])
                gt = gp.tile([128, 128], BF16, name=f"gst{tbg}")
                with nc.allow_low_precision(reason="gproj bf16 for expand"):
                    nc.vector.tensor_copy(gt[:], ps[:, 0:128])
                gstage.append(gt)

            # expand dg x4 into natural token blocks: dgx [128, 32]
            ps_dgx = psD.tile([128, 32], FP32, name="ps_dgx")
            for tb in range(32):
                base = 64 * ((tb % 4) // 2)
                rep = repA_sb if tb % 2 == 0 else repB_sb
                nc.tensor.matmul(ps_dgx[:, tb:tb + 1],
                                 rep[base:base + 64, :],
                                 dg_sb[base:base + 64, tb // 4:tb // 4 + 1],
                                 start=True, stop=True, skip_group_check=True)

            # local proj: psum -> outstage + dl column
            dl_sb = gp.tile([128, 32], FP32, name="dl_sb")
            outst = []
            for tb in range(32):
                ps = psP.tile([128, 129], FP32, name="psPl_t")
                for ch in range(4):
                    nc.tensor.matmul(
                        ps[:], att_all[ch][:, 128 * tb:128 * tb + 128],
                        wpl_sb[ch][:], start=(ch == 0), stop=False)
                nc.tensor.matmul(ps[:], ones_bf[:, 0:128], bple_sb[:],
                                 start=False, stop=True)
                nc.vector.tensor_copy(dl_sb[:, tb:tb + 1], ps[:, 128:129])
                ot = stp.tile([128, 128], FP32, name=f"outst{tb}")
                nc.scalar.activation(ot[:], ps[:, 0:128], AF.Copy)
                outst.append(ot)

            # gate: tanh(0.5*(dl+dgx) + 0.5*c0)
            dsum = gp.tile([128, 32], FP32, name="dsum")
            nc.vector.tensor_add(dsum[:], dl_sb[:], ps_dgx[:])
            tanh_sb = gp.tile([128, 32], FP32, name="tanh_sb")
            nc.scalar.activation(tanh_sb[:], dsum[:], AF.Tanh,
                                 scale=0.5, bias=biases["c0h"][:])
            g0 = gp.tile([128, 32], FP32, name="g0")
            g1 = gp.tile([128, 32], FP32, name="g1")
            nc.vector.tensor_scalar(g0[:], tanh_sb[:], 0.5, 0.5,
                                    ALU.mult, ALU.add)
            nc.vector.tensor_scalar(g1[:], tanh_sb[:], -0.5, 0.5,
                                    ALU.mult, ALU.add)

            for tb in range(32):
                o = outp.tile([128, 128], FP32, name="outl")
                nc.vector.tensor_scalar_mul(o[:], outst[tb][:],
                                            g0[:, tb:tb + 1])
                nc.sync.dma_start(out=out_loc[128 * tb:128 * tb + 128, :],
                                  in_=o[:])
            for tb in range(32):
                ps = psE.tile([128, 128], FP32, name="psE_t")
                base = 64 * ((tb % 4) // 2)
                rep = repA_sb if tb % 2 == 0 else repB_sb
                nc.tensor.matmul(ps[:], rep[base:base + 64, :],
                                 gstage[tb // 4][base:base + 64, :],
                                 start=True, stop=True)
                o = outp.tile([128, 128], FP32, name="outg")
                nc.vector.tensor_scalar_mul(o[:], ps[:], g1[:, tb:tb + 1])
                nc.sync.dma_start(out=out_glob[128 * tb:128 * tb + 128, :],
                                  in_=o[:])

    nc.finalize()
    return nc


# ---------------------------------------------------------------------------
# Host side
# ---------------------------------------------------------------------------

_NC_CACHE = []


def _get_program():
    if not _NC_CACHE:
        _NC_CACHE.append(build_program())
    return _NC_CACHE[0]


def _prep_inputs(x, w_lqkv, b_lqkv, w_gqkv, b_gqkv, w_comp, b_comp,
                 w_lproj, b_lproj, w_gproj, b_gproj, w_gate, b_gate):
    import ml_dtypes
    f32 = np.float32
    bf16 = ml_dtypes.bfloat16
    wd = (w_gate[:, 0] - w_gate[:, 1]).astype(f32)
    u_l = (w_lproj @ wd[:LD]).astype(f32)
    u_g = (w_gproj @ wd[LD:]).astype(f32)
    c0 = float(b_lproj @ wd[:LD] + b_gproj @ wd[LD:] + b_gate[0] - b_gate[1])

    mask_tri = np.where(np.arange(128)[None, :] >= np.arange(128)[:, None],
                        0.0, NEG).astype(f32)
    e0 = np.zeros((64, 128), f32)
    e0[np.arange(128) // 4, np.arange(128)] = 1.0
    e1 = np.zeros((64, 128), f32)
    e1[32 + np.arange(128) // 4, np.arange(128)] = 1.0
    repA_ = np.concatenate([e0, e0], axis=0)
    repB_ = np.concatenate([e1, e1], axis=0)
    sel2_ = np.zeros((2, 128), f32)
    sel2_[0, 0:64] = 1.0
    sel2_[1, 64:128] = 1.0

    def qk_packed(wqkv, bqkv, ha, hb):
        wA = np.concatenate([wqkv[:, D * ha:D * ha + D] / 8.0,
                             wqkv[:, D * hb:D * hb + D] / 8.0], axis=1)
        bA = np.concatenate([bqkv[D * ha:D * ha + D] / 8.0,
                             bqkv[D * hb:D * hb + D] / 8.0])
        wB = np.concatenate([wqkv[:, LD + D * ha:LD + D * ha + D],
                             wqkv[:, LD + D * hb:LD + D * hb + D]], axis=1)
        bB = np.concatenate([bqkv[LD + D * ha:LD + D * ha + D],
                             bqkv[LD + D * hb:LD + D * hb + D]])
        return (wA.reshape(4, 128, 128), bA.astype(f32).reshape(128, 1),
                wB.reshape(4, 128, 128), bB.astype(f32).reshape(128, 1))

    def v_packed(wqkv, bqkv, ha, hb):
        wv_ = np.concatenate([wqkv[:, 2 * LD + D * ha:2 * LD + D * ha + D],
                              wqkv[:, 2 * LD + D * hb:2 * LD + D * hb + D]],
                             axis=1)
        bv_ = np.concatenate([bqkv[2 * LD + D * ha:2 * LD + D * ha + D],
                              bqkv[2 * LD + D * hb:2 * LD + D * hb + D]])
        return wv_.reshape(4, 128, 128), bv_.astype(f32).reshape(1, 128)

    in_maps = []
    for core in range(NCORES):
        b_idx, g = core // 4, core % 4
        ha, hb = 2 * g, 2 * g + 1
        cs = slice(128 * g, 128 * g + 128)

        x2t_ = np.ascontiguousarray(x[b_idx].T).reshape(8, 128, T)
        wc_s = np.ascontiguousarray(
            w_comp[:, LD + 128 * g:LD + 128 * g + 128]).reshape(32, 128, 128)
        bc_s = b_comp[LD + 128 * g:LD + 128 * g + 128].astype(f32) \
            .reshape(128, 1)

        wqkA_, bqkA_, wqkB_, bqkB_ = qk_packed(w_lqkv, b_lqkv, ha, hb)
        wv_, bvr_ = v_packed(w_lqkv, b_lqkv, ha, hb)
        wgqkA_, bgqkA_, wgqkB_, bgqkB_ = qk_packed(w_gqkv, b_gqkv, ha, hb)
        wgv_, bgvr_ = v_packed(w_gqkv, b_gqkv, ha, hb)

        wpl_ = np.concatenate([w_lproj[:, cs], u_l[:, None]],
                              axis=1).reshape(4, 128, 129)
        wpg_ = np.concatenate([w_gproj[:, cs], u_g[:, None]],
                              axis=1).reshape(4, 128, 129)
        bple_ = np.concatenate([b_lproj[cs], [0.0]]).astype(f32) \
            .reshape(1, 129)
        bpge_ = np.concatenate([b_gproj[cs], [0.0]]).astype(f32) \
            .reshape(1, 129)

        m = {
            "x2t": x2t_, "wc": wc_s, "bc": bc_s,
            "wqkA": wqkA_, "bqkA": bqkA_, "wqkB": wqkB_, "bqkB": bqkB_,
            "wv": wv_, "bvr": bvr_,
            "wgqkA": wgqkA_, "bgqkA": bgqkA_, "wgqkB": wgqkB_,
            "bgqkB": bgqkB_, "wgv": wgv_, "bgvr": bgvr_,
            "wpl": wpl_, "wpg": wpg_, "bple": bple_, "bpge": bpge_,
            "c0h": np.full((128, 1), 0.5 * c0, f32),
            "onesr": np.ones((1, 129), f32),
            "onescb": np.ones((128, 1), f32),
            "repA": repA_, "repB": repB_, "maskt": mask_tri,
        }
        for k in ("x2t", "wc", "wqkA", "wqkB", "wv", "bvr", "wgqkA", "wgqkB",
                  "wgv", "bgvr", "wpl", "wpg", "bple", "bpge", "onesr",
                  "onescb", "repA", "repB", "maskt"):
            m[k] = m[k].astype(bf16)
        in_maps.append(m)
    return in_maps


def _run(in_maps, trace=False):
    nc = _get_program()
    return run_bass_kernel_spmd(nc, in_maps, list(range(NCORES)), trace=trace)


def assemble(results):
    out = np.empty((B, T, E), np.float32)
    for core in range(NCORES):
        b_idx, g = core // 4, core % 4
        out[b_idx, :, 128 * g:128 * g + 128] = results[core]["out_loc"]
        out[b_idx, :, LD + 128 * g:LD + 128 * g + 128] = \
            results[core]["out_glob"]
    return out


def kernel(**inputs):
    in_maps = _prep_inputs(**inputs)
    res = _run(in_maps)
    return assemble(res.results)


def kernel_traced(**inputs):
    in_maps = _prep_inputs(**inputs)
    res = _run(in_maps, trace=True)
    return assemble(res.results), res



# revision 33
# speedup vs baseline: 1.1756x; 1.0203x over previous
"""DualResolutionAttention Trainium2 kernel v3 (8 NeuronCores, Bass/Tile).

Sharding: core c -> (batch b = c//4, group g = c%4); heads {2g, 2g+1} both
branches; output channel slice [128g, 128g+128) of each branch.

v3 vs v2:
- whole datapath bf16 (x input shipped bf16): LDWEIGHTS pipelines (no fp32r
  self-load tax), input DMA halves to 8MB
- attention: 3-way rotating [128,1024] score psum tiles + per-head exp ->
  scalar engine stays saturated (no scores->exp serialization)
- PV col-tiled packed: h0 -> psum partitions 0:64, h1 -> 64:128, one
  [128,1024] accumulator (2 banks)
- softmax denominators: DVE-accumulated den_acc (bf16) + ones-column matmul
  partition-reduce; no ones-column in v, no stag reshape machinery
- compress split into 4 quarter-chunks run at strip boundaries, accumulated
  in SBUF via DVE adds; cg AllGather overlaps strips 2-3
"""
import os
import sys

sys.path.insert(0, "/opt/trn_rl_repo")
os.environ.setdefault("JAX_PLATFORMS", "axon,cpu")

from contextlib import ExitStack

import numpy as np

import concourse.bass as bass
import concourse.mybir as mybir
import concourse.tile as tile
from concourse import bacc
from concourse.bass_utils import run_bass_kernel_spmd
from concourse.masks import make_identity

FP32 = mybir.dt.float32
FP32R = mybir.dt.float32r
BF16 = mybir.dt.bfloat16
AF = mybir.ActivationFunctionType
ALU = mybir.AluOpType

B, T, E = 2, 4096, 1024
LD = 512
D = 64
R = 4
Tc = T // R
NCORES = 8
GROUPS = [[0, 1, 2, 3], [4, 5, 6, 7]]
NEG = -1.0e9


def build_program():
    nc = bacc.Bacc(None, target_bir_lowering=False)

    def inp(name, shape, dt=BF16):
        return nc.declare_dram_parameter(name, list(shape), dt, isOutput=False)

    x2t = inp("x2t", [8, 128, T])            # x[b].T chunks, bf16
    wc = inp("wc", [32, 128, 128])           # compress slice lhsT chunks
    bc = inp("bc", [128, 1], FP32)
    wqkA = inp("wqkA", [4, 128, 128])        # [q_h0/8 | q_h1/8] lhsT chunks
    wqkB = inp("wqkB", [4, 128, 128])        # [k_h0 | k_h1] lhsT chunks
    bqkA = inp("bqkA", [128, 1], FP32)
    bqkB = inp("bqkB", [128, 1], FP32)
    wv = inp("wv", [4, 128, 128])            # [v_h0 | v_h1] rhs chunks
    bvr = inp("bvr", [1, 128])               # [bv_h0 | bv_h1] bias row
    wgqkA = inp("wgqkA", [4, 128, 128])
    wgqkB = inp("wgqkB", [4, 128, 128])
    bgqkA = inp("bgqkA", [128, 1], FP32)
    bgqkB = inp("bgqkB", [128, 1], FP32)
    wgv = inp("wgv", [4, 128, 128])
    bgvr = inp("bgvr", [1, 128])
    wpl = inp("wpl", [4, 128, 129])          # [w_lproj slice | u_l] chunks
    wpg = inp("wpg", [4, 128, 129])
    bple = inp("bple", [1, 129])             # [b_lproj slice, 0]
    bpge = inp("bpge", [1, 129])
    c0h = inp("c0h", [128, 1], FP32)         # 0.5 * gate const
    onesr = inp("onesr", [1, 129])           # ones row (bf16)
    onescb = inp("onescb", [128, 1])         # ones column (bf16)
    repA = inp("repA", [128, 128])           # x4 expander (even 32-blocks)
    repB = inp("repB", [128, 128])
    maskt = inp("maskt", [128, 128])         # strict lower-tri NEG
    out_loc = nc.declare_dram_parameter("out_loc", [T, 128], FP32, isOutput=True)
    out_glob = nc.declare_dram_parameter("out_glob", [T, 128], FP32,
                                         isOutput=True)

    with tile.TileContext(nc) as tc:
      with ExitStack() as top:
        dram = top.enter_context(tc.tile_pool(name="dram", bufs=1, space="DRAM"))
        const = top.enter_context(tc.tile_pool(name="const", bufs=1))
        persist = top.enter_context(tc.tile_pool(name="persist", bufs=1))

        # ---- constants
        ident_b = const.tile([128, 128], BF16, name="ident_b")
        make_identity(nc, ident_b[:])
        mask_tri = const.tile([128, 128], BF16, name="mask_tri")
        nc.sync.dma_start(out=mask_tri[:], in_=maskt[:])
        repA_sb = const.tile([128, 128], BF16, name="repA_sb")
        nc.sync.dma_start(out=repA_sb[:], in_=repA[:])
        repB_sb = const.tile([128, 128], BF16, name="repB_sb")
        nc.sync.dma_start(out=repB_sb[:], in_=repB[:])
        onescb_sb = const.tile([128, 1], BF16, name="onescb_sb")
        nc.sync.dma_start(out=onescb_sb[:], in_=onescb[:])
        ones_f = const.tile([1, 64], FP32, name="ones_f")
        nc.vector.memset(ones_f[:], 1.0)
        ones_bf = const.tile([1, 129], BF16, name="ones_bf")
        nc.sync.dma_start(out=ones_bf[:], in_=onesr[:])
        bple_sb = const.tile([1, 129], BF16, name="bple_sb")
        nc.sync.dma_start(out=bple_sb[:], in_=bple[:])
        bpge_sb = const.tile([1, 129], BF16, name="bpge_sb")
        nc.sync.dma_start(out=bpge_sb[:], in_=bpge[:])
        bvr_sb = const.tile([1, 128], BF16, name="bvr_sb")
        nc.sync.dma_start(out=bvr_sb[:], in_=bvr[:])
        bgvr_sb = const.tile([1, 128], BF16, name="bgvr_sb")
        nc.sync.dma_start(out=bgvr_sb[:], in_=bgvr[:])
        biases = {}
        for nm, src in (("bc", bc), ("c0h", c0h),
                        ("bqkA", bqkA), ("bqkB", bqkB),
                        ("bgqkA", bgqkA), ("bgqkB", bgqkB)):
            t = const.tile([128, 1], FP32, name=f"cb_{nm}")
            nc.sync.dma_start(out=t[:], in_=src[:])
            biases[nm] = t

        # ---- persistent tensors (all bf16)
        qT_l = persist.tile([128, T], BF16, name="qT_l")
        kT_l = persist.tile([128, T], BF16, name="kT_l")
        qT_g = persist.tile([128, Tc], BF16, name="qT_g")
        kT_g = persist.tile([128, Tc], BF16, name="kT_g")
        v_sb_l = [persist.tile([128, 128], BF16, name=f"vsb{i}")
                  for i in range(32)]
        v_sb_g = [persist.tile([128, 128], BF16, name=f"vgsb{i}")
                  for i in range(8)]
        cg_all = [persist.tile([128, Tc], BF16, name=f"cg_all{i}")
                  for i in range(4)]
        cgT32 = persist.tile([128, Tc], FP32, name="cgT32")

        # DRAM bounce tiles for collectives
        cg_contrib = dram.tile([128, Tc], BF16, name="cg_contrib")
        cg_gathered = dram.tile([512, Tc], BF16, name="cg_gathered")
        attl_contrib = [dram.tile([128, 1024], BF16, name=f"alc{s}")
                        for s in range(4)]
        attl_gathered = [dram.tile([512, 1024], BF16, name=f"alg{s}")
                         for s in range(4)]
        attg_contrib = dram.tile([128, Tc], BF16, name="agc")
        attg_gathered = dram.tile([512, Tc], BF16, name="agg")

        ap_pool = top.enter_context(tc.tile_pool(name="attall", bufs=1))
        att_all = [ap_pool.tile([128, T], BF16, name=f"attall{ch}")
                   for ch in range(4)]
        attg_all = [ap_pool.tile([128, Tc], BF16, name=f"attgall{ch}")
                    for ch in range(4)]

        # =========================================== phase AB: x load, qkv, v
        pab = top.enter_context(ExitStack())
        xq_pool = pab.enter_context(tc.tile_pool(name="xq_pool", bufs=1))
        xs_pool = pab.enter_context(tc.tile_pool(name="xs_pool", bufs=8))
        w_pool = pab.enter_context(tc.tile_pool(name="w_pool", bufs=1))
        wcp = pab.enter_context(tc.tile_pool(name="wc_pool", bufs=8))

        x_lo = []
        for cc in range(4):
            t = xq_pool.tile([128, T], BF16, name=f"x2t{cc}")
            nc.scalar.dma_start(out=t[:], in_=x2t[cc])
            x_lo.append(t)

        wqkA_sb, wqkB_sb, wv_sb = [], [], []
        for cc in range(4):
            t = w_pool.tile([128, 128], BF16, name=f"wqkA{cc}")
            nc.sync.dma_start(out=t[:], in_=wqkA[cc])
            wqkA_sb.append(t)
            t = w_pool.tile([128, 128], BF16, name=f"wqkB{cc}")
            nc.sync.dma_start(out=t[:], in_=wqkB[cc])
            wqkB_sb.append(t)
            t = w_pool.tile([128, 128], BF16, name=f"wv{cc}")
            nc.sync.dma_start(out=t[:], in_=wv[cc])
            wv_sb.append(t)

        with ExitStack() as pqkv:
            psAB = pqkv.enter_context(
                tc.tile_pool(name="psAB", bufs=2, space="PSUM"))
            psV = pqkv.enter_context(
                tc.tile_pool(name="psV", bufs=2, space="PSUM"))
            # local q/k: packed tiles -> direct DVE copy
            for (wts, bias_ap, dst) in ((wqkA_sb, biases["bqkA"][:], qT_l),
                                        (wqkB_sb, biases["bqkB"][:], kT_l)):
                for qt in range(8):
                    ps = psAB.tile([128, 512], FP32, name="psAB_t")
                    for cc in range(4):
                        nc.tensor.matmul(
                            ps[:], wts[cc][:],
                            x_lo[cc][:, 512 * qt:512 * qt + 512],
                            start=(cc == 0), stop=(cc == 3))
                    with nc.allow_low_precision(reason="qk bf16"):
                        nc.scalar.activation(dst[:, 512 * qt:512 * qt + 512],
                                             ps[:], AF.Identity, bias=bias_ap)
            # local v: x-stationary, v_sb[t, d] direct
            for kb in range(32):
                ps = psV.tile([128, 128], FP32, name="psV_t")
                for cc in range(4):
                    nc.tensor.matmul(
                        ps[:], x_lo[cc][:, 128 * kb:128 * kb + 128],
                        wv_sb[cc][:], start=(cc == 0), stop=False)
                nc.tensor.matmul(ps[:], ones_bf[:, 0:128], bvr_sb[:],
                                 start=False, stop=True)
                with nc.allow_low_precision(reason="v bf16"):
                    nc.vector.tensor_copy(v_sb_l[kb][:], ps[:])

        # =============================================== attention machinery
        def attention_strip(pools, nkb, q0, qT, kT, v_sb):
            """One 1024-q strip, both heads; returns (hold, den_sb[2])."""
            s_pool, o_pool, p_pool, hold_pool, den_pool, dn_pool = pools
            psum_o = o_pool.tile([128, 1024], FP32, name="ps_o")
            den_acc = [den_pool.tile([128, 1024], BF16, name="den_t")
                       for _ in range(2)]
            pend = None

            def emit_pv(j, ps_list, t0):
                for h in range(2):
                    for qs in (0, 512):
                        lo = max(qs, t0)
                        hi = qs + 512
                        if lo >= hi:
                            continue
                        nc.tensor.matmul(
                            psum_o[64 * h:64 * h + 64, lo:hi],
                            v_sb[j][:, 64 * h:64 * h + 64],
                            ps_list[h][:, lo:hi],
                            start=(j == 0), stop=(j == nkb - 1),
                            skip_group_check=True)

            for j in range(nkb):
                t0 = max(0, 128 * j - q0)
                diag = 128 * j >= q0
                p2 = []
                for h in range(2):
                    ps = s_pool.tile([128, 1024], FP32, name="ps_s")
                    for qs in (0, 512):
                        if qs + 512 <= (t0 // 512) * 512:
                            continue
                        nc.tensor.matmul(
                            ps[:, qs:qs + 512],
                            kT[64 * h:64 * h + 64, 128 * j:128 * j + 128],
                            qT[64 * h:64 * h + 64, q0 + qs:q0 + qs + 512],
                            start=True, stop=True)
                    if diag:
                        nc.tensor.matmul(
                            ps[:, t0:t0 + 128], ident_b[:], mask_tri[:],
                            start=False, stop=True, skip_group_check=True)
                    p = p_pool.tile([128, 1024], BF16, name="p_t")
                    with nc.allow_low_precision(reason="softmax p bf16"):
                        nc.scalar.activation(p[:, t0:1024], ps[:, t0:1024],
                                             AF.Exp)
                    with nc.allow_low_precision(reason="den bf16"):
                        if j == 0:
                            nc.vector.tensor_copy(den_acc[h][:], p[:])
                        else:
                            nc.vector.tensor_add(den_acc[h][:, t0:1024],
                                                 den_acc[h][:, t0:1024],
                                                 p[:, t0:1024])
                    p2.append(p)
                if pend is not None:
                    emit_pv(*pend)
                pend = (j, p2, t0)
            emit_pv(*pend)

            # denominator partition-reduce + 1-lane copies
            den_sb = []
            for h in range(2):
                psd = s_pool.tile([128, 1024], FP32, name="ps_s")
                for qs in (0, 512):
                    nc.tensor.matmul(psd[0:1, qs:qs + 512], onescb_sb[:],
                                     den_acc[h][:, qs:qs + 512],
                                     start=True, stop=True,
                                     skip_group_check=True)
                d = dn_pool.tile([1, 1024], FP32, name="dn_t")
                nc.vector.tensor_copy(d[:], psd[0:1, :])
                den_sb.append(d)
            hold = hold_pool.tile([128, 1024], FP32, name="hold_t")
            nc.vector.tensor_copy(hold[:], psum_o[:])
            return hold, den_sb

        def normalize_strip(pools, hold, den_sb, contrib, gathered):
            s_pool, abp, rpp = pools
            recs = []
            for h in range(2):
                rc32 = rpp.tile([1, 1024], FP32, name="rec32_t")
                nc.vector.reciprocal_approx_fast(out=rc32[:],
                                                 in_=den_sb[h][:])
                rc = rpp.tile([1, 1024], BF16, name="rec_t")
                with nc.allow_low_precision(reason="recip bf16"):
                    nc.vector.tensor_copy(rc[:], rc32[:])
                recs.append(rc)
            psw = s_pool.tile([128, 1024], FP32, name="ps_s")
            for h in range(2):
                for qs in (0, 512):
                    nc.tensor.matmul(
                        psw[64 * h:64 * h + 64, qs:qs + 512],
                        ones_bf[0:1, 0:64], recs[h][:, qs:qs + 512],
                        start=True, stop=True, skip_group_check=True)
            ab = abp.tile([128, 1024], BF16, name="ab_t")
            with nc.allow_low_precision(reason="attnorm bf16"):
                nc.vector.tensor_mul(ab[:], hold[:], psw[:])
            nc.sync.dma_start(out=contrib[:], in_=ab[:])
            nc.gpsimd.collective_compute(
                "AllGather", ALU.bypass, replica_groups=GROUPS,
                ins=[contrib.opt()], outs=[gathered.opt()])

        # =============================================== attention + compress
        with ExitStack() as pc:
            s_pool = pc.enter_context(
                tc.tile_pool(name="s_pool", bufs=3, space="PSUM"))
            o_pool = pc.enter_context(
                tc.tile_pool(name="o_pool", bufs=1, space="PSUM"))
            p_pool = pc.enter_context(tc.tile_pool(name="p_pool", bufs=4))
            hold_pool = pc.enter_context(tc.tile_pool(name="hold", bufs=2))
            den_pool = pc.enter_context(tc.tile_pool(name="den", bufs=4))
            dn_pool = pc.enter_context(tc.tile_pool(name="dn", bufs=2))
            abp = pc.enter_context(tc.tile_pool(name="abp", bufs=2))
            att_pools = (s_pool, o_pool, p_pool, hold_pool, den_pool, dn_pool)
            norm_pools = (s_pool, abp, pc.enter_context(
                tc.tile_pool(name="rpp", bufs=2)))

            # prefetch high x chunks (4..7) as halves; they stream during
            # strips 0-1 and feed compress quarters 2-3
            x_hi = {}
            for cc in range(4, 8):
                h0 = xs_pool.tile([128, 2048], BF16, name="xs_t")
                nc.scalar.dma_start(out=h0[:], in_=x2t[cc][:, 0:2048])
                h1 = xs_pool.tile([128, 2048], BF16, name="xs_t")
                nc.scalar.dma_start(out=h1[:], in_=x2t[cc][:, 2048:4096])
                x_hi[cc] = (h0, h1)

            def compress_quarters(quarters):
                # one quarter: chunk-pair (2cq, 2cq+1) x 4 phases, both halves
                for cq in quarters:
                    psw = s_pool.tile([128, 1024], FP32, name="ps_s")
                    n = 0
                    for cc in (2 * cq, 2 * cq + 1):
                        if cc < 4:
                            srcs = [(x_lo[cc], 0), (x_lo[cc], 2048)]
                        else:
                            srcs = [(x_hi[cc][0], 0), (x_hi[cc][1], 0)]
                        for r in range(4):
                            w = wcp.tile([128, 128], BF16, name="wc_t")
                            nc.sync.dma_start(out=w[:], in_=wc[8 * r + cc])
                            for hf in range(2):
                                xh, off = srcs[hf]
                                nc.tensor.matmul(
                                    psw[:, 512 * hf:512 * hf + 512], w[:],
                                    xh[:, off + r:off + 2048:4],
                                    start=(n == 0), stop=(n == 7))
                            n += 1
                    if cq == 0:
                        nc.vector.tensor_copy(cgT32[:], psw[:])
                    else:
                        nc.vector.tensor_add(cgT32[:], cgT32[:], psw[:])

            holds_l, dens_l = [], []
            for s in range(4):
                h, d = attention_strip(att_pools, 8 * s + 8, 1024 * s,
                                       qT_l, kT_l, v_sb_l)
                normalize_strip(norm_pools, h, d, attl_contrib[s],
                                attl_gathered[s])
                for ch in range(4):
                    nc.sync.dma_start(
                        out=att_all[ch][:, 1024 * s:1024 * s + 1024],
                        in_=attl_gathered[s][128 * ch:128 * ch + 128, :])
                if s == 0:
                    compress_quarters([0, 1])
                elif s == 1:
                    compress_quarters([2, 3])
                    # finalize cg: bias + bf16 cast, AllGather
                    cg_bf = abp.tile([128, 1024], BF16, name="ab_t")
                    with nc.allow_low_precision(reason="cg bf16"):
                        nc.scalar.activation(cg_bf[:], cgT32[:], AF.Identity,
                                             bias=biases["bc"][:])
                    nc.sync.dma_start(out=cg_contrib[:], in_=cg_bf[:])
                    nc.gpsimd.collective_compute(
                        "AllGather", ALU.bypass, replica_groups=GROUPS,
                        ins=[cg_contrib.opt()], outs=[cg_gathered.opt()])
                    for i in range(4):
                        nc.sync.dma_start(
                            out=cg_all[i][:],
                            in_=cg_gathered[128 * i:128 * i + 128, :])
                elif s == 2:
                    # global qkv + v from gathered cg
                    with ExitStack() as pg:
                        wgp = pg.enter_context(
                            tc.tile_pool(name="wg_pool", bufs=1))
                        wgqkA_sb, wgqkB_sb, wgv_sb = [], [], []
                        for cc in range(4):
                            t = wgp.tile([128, 128], BF16, name=f"wgqkA{cc}")
                            nc.sync.dma_start(out=t[:], in_=wgqkA[cc])
                            wgqkA_sb.append(t)
                            t = wgp.tile([128, 128], BF16, name=f"wgqkB{cc}")
                            nc.sync.dma_start(out=t[:], in_=wgqkB[cc])
                            wgqkB_sb.append(t)
                            t = wgp.tile([128, 128], BF16, name=f"wgv{cc}")
                            nc.sync.dma_start(out=t[:], in_=wgv[cc])
                            wgv_sb.append(t)
                        for (wts, bias_ap, dst) in (
                                (wgqkA_sb, biases["bgqkA"][:], qT_g),
                                (wgqkB_sb, biases["bgqkB"][:], kT_g)):
                            for qt in range(2):
                                psw = s_pool.tile([128, 1024], FP32,
                                                  name="ps_s")
                                ps = psw[:, 0:512]
                                for cc in range(4):
                                    nc.tensor.matmul(
                                        ps, wts[cc][:],
                                        cg_all[cc][:, 512 * qt:512 * qt + 512],
                                        start=(cc == 0), stop=(cc == 3))
                                with nc.allow_low_precision(reason="gqk bf16"):
                                    nc.scalar.activation(
                                        dst[:, 512 * qt:512 * qt + 512],
                                        ps, AF.Identity, bias=bias_ap)
                        for kb in range(8):
                            psw = s_pool.tile([128, 1024], FP32, name="ps_s")
                            ps = psw[:, 0:128]
                            for cc in range(4):
                                nc.tensor.matmul(
                                    ps, cg_all[cc][:, 128 * kb:128 * kb + 128],
                                    wgv_sb[cc][:], start=(cc == 0), stop=False)
                            nc.tensor.matmul(ps, ones_bf[:, 0:128],
                                             bgvr_sb[:],
                                             start=False, stop=True)
                            with nc.allow_low_precision(reason="gv bf16"):
                                nc.vector.tensor_copy(v_sb_g[kb][:], ps)
                    # global attention here so its AllGather overlaps strip 3
                    gh, gd = attention_strip(att_pools, 8, 0,
                                             qT_g, kT_g, v_sb_g)
                    normalize_strip(norm_pools, gh, gd, attg_contrib,
                                    attg_gathered)
                    for ch in range(4):
                        nc.sync.dma_start(
                            out=attg_all[ch][:],
                            in_=attg_gathered[128 * ch:128 * ch + 128, :])

        pab.close()

        # =============================================== proj + gate + out
        with ExitStack() as pd:
            wpp = pd.enter_context(tc.tile_pool(name="wp_pool", bufs=1))
            psP = pd.enter_context(tc.tile_pool(name="psP", bufs=2,
                                                space="PSUM"))
            psE = pd.enter_context(tc.tile_pool(name="psE", bufs=2,
                                                space="PSUM"))
            psD = pd.enter_context(tc.tile_pool(name="psD", bufs=1,
                                                space="PSUM"))
            gp = pd.enter_context(tc.tile_pool(name="g_pool", bufs=1))
            outp = pd.enter_context(tc.tile_pool(name="out_pool", bufs=4))
            stp = pd.enter_context(tc.tile_pool(name="stage_pool", bufs=1))

            wpl_sb, wpg_sb = [], []
            for ch in range(4):
                t = wpp.tile([128, 129], BF16, name=f"wpl{ch}")
                nc.sync.dma_start(out=t[:], in_=wpl[ch])
                wpl_sb.append(t)
                t = wpp.tile([128, 129], BF16, name=f"wpg{ch}")
                nc.sync.dma_start(out=t[:], in_=wpg[ch])
                wpg_sb.append(t)

            # global proj first (its gate column feeds the local gate)
            dg_sb = gp.tile([128, 8], BF16, name="dg_sb")
            gstage = []
            for tbg in range(8):
                ps = psP.tile([128, 129], FP32, name="psPg_t")
                for ch in range(4):
                    nc.tensor.matmul(
                        ps[:], attg_all[ch][:, 128 * tbg:128 * tbg + 128],
                        wpg_sb[ch][:], start=(ch == 0), stop=False)
                nc.tensor.matmul(ps[:], ones_bf[:, 0:128], bpge_sb[:],
                                 start=False, stop=True)
                with nc.allow_low_precision(reason="gate logit bf16"):
                    nc.vector.tensor_copy(dg_sb[:, tbg:tbg + 1],
                                          ps[:, 128:129])
                gt = gp.tile([128, 128], BF16, name=f"gst{tbg}")
                with nc.allow_low_precision(reason="gproj bf16 for expand"):
                    nc.vector.tensor_copy(gt[:], ps[:, 0:128])
                gstage.append(gt)

            # expand dg x4 into natural token blocks: dgx [128, 32]
            ps_dgx = psD.tile([128, 32], FP32, name="ps_dgx")
            for tb in range(32):
                base = 64 * ((tb % 4) // 2)
                rep = repA_sb if tb % 2 == 0 else repB_sb
                nc.tensor.matmul(ps_dgx[:, tb:tb + 1],
                                 rep[base:base + 64, :],
                                 dg_sb[base:base + 64, tb // 4:tb // 4 + 1],
                                 start=True, stop=True, skip_group_check=True)

            # local proj: psum -> outstage + dl column
            dl_sb = gp.tile([128, 32], FP32, name="dl_sb")
            outst = []
            for tb in range(32):
                ps = psP.tile([128, 129], FP32, name="psPl_t")
                for ch in range(4):
                    nc.tensor.matmul(
                        ps[:], att_all[ch][:, 128 * tb:128 * tb + 128],
                        wpl_sb[ch][:], start=(ch == 0), stop=False)
                nc.tensor.matmul(ps[:], ones_bf[:, 0:128], bple_sb[:],
                                 start=False, stop=True)
                nc.vector.tensor_copy(dl_sb[:, tb:tb + 1], ps[:, 128:129])
                ot = stp.tile([128, 128], FP32, name=f"outst{tb}")
                nc.scalar.activation(ot[:], ps[:, 0:128], AF.Copy)
                outst.append(ot)

            # gate: tanh(0.5*(dl+dgx) + 0.5*c0)
            dsum = gp.tile([128, 32], FP32, name="dsum")
            nc.vector.tensor_add(dsum[:], dl_sb[:], ps_dgx[:])
            tanh_sb = gp.tile([128, 32], FP32, name="tanh_sb")
            nc.scalar.activation(tanh_sb[:], dsum[:], AF.Tanh,
                                 scale=0.5, bias=biases["c0h"][:])
            g0 = gp.tile([128, 32], FP32, name="g0")
            g1 = gp.tile([128, 32], FP32, name="g1")
            nc.vector.tensor_scalar(g0[:], tanh_sb[:], 0.5, 0.5,
                                    ALU.mult, ALU.add)
            nc.vector.tensor_scalar(g1[:], tanh_sb[:], -0.5, 0.5,
                                    ALU.mult, ALU.add)

            for tb in range(32):
                o = outp.tile([128, 128], FP32, name="outl")
                nc.vector.tensor_scalar_mul(o[:], outst[tb][:],
                                            g0[:, tb:tb + 1])
                nc.sync.dma_start(out=out_loc[128 * tb:128 * tb + 128, :],
                                  in_=o[:])
            for tb in range(32):
                ps = psE.tile([128, 128], FP32, name="psE_t")
                base = 64 * ((tb % 4) // 2)
                rep = repA_sb if tb % 2 == 0 else repB_sb
                nc.tensor.matmul(ps[:], rep[base:base + 64, :],
                                 gstage[tb // 4][base:base + 64, :],
                                 start=True, stop=True)
                o = outp.tile([128, 128], FP32, name="outg")
                nc.vector.tensor_scalar_mul(o[:], ps[:], g1[:, tb:tb + 1])
                nc.sync.dma_start(out=out_glob[128 * tb:128 * tb + 128, :],
                                  in_=o[:])

    nc.finalize()
    return nc


# ---------------------------------------------------------------------------
# Host side
# ---------------------------------------------------------------------------

_NC_CACHE = []


def _get_program():
    if not _NC_CACHE:
        _NC_CACHE.append(build_program())
    return _NC_CACHE[0]


def _prep_inputs(x, w_lqkv, b_lqkv, w_gqkv, b_gqkv, w_comp, b_comp,
                 w_lproj, b_lproj, w_gproj, b_gproj, w_gate, b_gate):
    import ml_dtypes
    f32 = np.float32
    bf16 = ml_dtypes.bfloat16
    wd = (w_gate[:, 0] - w_gate[:, 1]).astype(f32)
    u_l = (w_lproj @ wd[:LD]).astype(f32)
    u_g = (w_gproj @ wd[LD:]).astype(f32)
    c0 = float(b_lproj @ wd[:LD] + b_gproj @ wd[LD:] + b_gate[0] - b_gate[1])

    mask_tri = np.where(np.arange(128)[None, :] >= np.arange(128)[:, None],
                        0.0, NEG).astype(f32)
    e0 = np.zeros((64, 128), f32)
    e0[np.arange(128) // 4, np.arange(128)] = 1.0
    e1 = np.zeros((64, 128), f32)
    e1[32 + np.arange(128) // 4, np.arange(128)] = 1.0
    repA_ = np.concatenate([e0, e0], axis=0)
    repB_ = np.concatenate([e1, e1], axis=0)
    sel2_ = np.zeros((2, 128), f32)
    sel2_[0, 0:64] = 1.0
    sel2_[1, 64:128] = 1.0

    def qk_packed(wqkv, bqkv, ha, hb):
        wA = np.concatenate([wqkv[:, D * ha:D * ha + D] / 8.0,
                             wqkv[:, D * hb:D * hb + D] / 8.0], axis=1)
        bA = np.concatenate([bqkv[D * ha:D * ha + D] / 8.0,
                             bqkv[D * hb:D * hb + D] / 8.0])
        wB = np.concatenate([wqkv[:, LD + D * ha:LD + D * ha + D],
                             wqkv[:, LD + D * hb:LD + D * hb + D]], axis=1)
        bB = np.concatenate([bqkv[LD + D * ha:LD + D * ha + D],
                             bqkv[LD + D * hb:LD + D * hb + D]])
        return (wA.reshape(4, 128, 128), bA.astype(f32).reshape(128, 1),
                wB.reshape(4, 128, 128), bB.astype(f32).reshape(128, 1))

    def v_packed(wqkv, bqkv, ha, hb):
        wv_ = np.concatenate([wqkv[:, 2 * LD + D * ha:2 * LD + D * ha + D],
                              wqkv[:, 2 * LD + D * hb:2 * LD + D * hb + D]],
                             axis=1)
        bv_ = np.concatenate([bqkv[2 * LD + D * ha:2 * LD + D * ha + D],
                              bqkv[2 * LD + D * hb:2 * LD + D * hb + D]])
        return wv_.reshape(4, 128, 128), bv_.astype(f32).reshape(1, 128)

    in_maps = []
    for core in range(NCORES):
        b_idx, g = core // 4, core % 4
        ha, hb = 2 * g, 2 * g + 1
        cs = slice(128 * g, 128 * g + 128)

        x2t_ = np.ascontiguousarray(x[b_idx].T).reshape(8, 128, T)
        wc_s = np.ascontiguousarray(
            w_comp[:, LD + 128 * g:LD + 128 * g + 128]).reshape(32, 128, 128)
        bc_s = b_comp[LD + 128 * g:LD + 128 * g + 128].astype(f32) \
            .reshape(128, 1)

        wqkA_, bqkA_, wqkB_, bqkB_ = qk_packed(w_lqkv, b_lqkv, ha, hb)
        wv_, bvr_ = v_packed(w_lqkv, b_lqkv, ha, hb)
        wgqkA_, bgqkA_, wgqkB_, bgqkB_ = qk_packed(w_gqkv, b_gqkv, ha, hb)
        wgv_, bgvr_ = v_packed(w_gqkv, b_gqkv, ha, hb)

        wpl_ = np.concatenate([w_lproj[:, cs], u_l[:, None]],
                              axis=1).reshape(4, 128, 129)
        wpg_ = np.concatenate([w_gproj[:, cs], u_g[:, None]],
                              axis=1).reshape(4, 128, 129)
        bple_ = np.concatenate([b_lproj[cs], [0.0]]).astype(f32) \
            .reshape(1, 129)
        bpge_ = np.concatenate([b_gproj[cs], [0.0]]).astype(f32) \
            .reshape(1, 129)

        m = {
            "x2t": x2t_, "wc": wc_s, "bc": bc_s,
            "wqkA": wqkA_, "bqkA": bqkA_, "wqkB": wqkB_, "bqkB": bqkB_,
            "wv": wv_, "bvr": bvr_,
            "wgqkA": wgqkA_, "bgqkA": bgqkA_, "wgqkB": wgqkB_,
            "bgqkB": bgqkB_, "wgv": wgv_, "bgvr": bgvr_,
            "wpl": wpl_, "wpg": wpg_, "bple": bple_, "bpge": bpge_,
            "c0h": np.full((128, 1), 0.5 * c0, f32),
            "onesr": np.ones((1, 129), f32),
            "onescb": np.ones((128, 1), f32),
            "repA": repA_, "repB": repB_, "maskt": mask_tri,
        }
        for k in ("x2t", "wc", "wqkA", "wqkB", "wv", "bvr", "wgqkA", "wgqkB",
                  "wgv", "bgvr", "wpl", "wpg", "bple", "bpge", "onesr",
                  "onescb", "repA", "repB", "maskt"):
            m[k] = m[k].astype(bf16)
        in_maps.append(m)
    return in_maps


def _run(in_maps, trace=False):
    nc = _get_program()
    return run_bass_kernel_spmd(nc, in_maps, list(range(NCORES)), trace=trace)


def assemble(results):
    out = np.empty((B, T, E), np.float32)
    for core in range(NCORES):
        b_idx, g = core // 4, core % 4
        out[b_idx, :, 128 * g:128 * g + 128] = results[core]["out_loc"]
        out[b_idx, :, LD + 128 * g:LD + 128 * g + 128] = \
            results[core]["out_glob"]
    return out


def kernel(**inputs):
    in_maps = _prep_inputs(**inputs)
    res = _run(in_maps)
    return assemble(res.results)


def kernel_traced(**inputs):
    in_maps = _prep_inputs(**inputs)
    res = _run(in_maps, trace=True)
    return assemble(res.results), res



# revision 34
# speedup vs baseline: 1.2067x; 1.0265x over previous
"""DualResolutionAttention Trainium2 kernel v3 (8 NeuronCores, Bass/Tile).

Sharding: core c -> (batch b = c//4, group g = c%4); heads {2g, 2g+1} both
branches; output channel slice [128g, 128g+128) of each branch.

v3 vs v2:
- whole datapath bf16 (x input shipped bf16): LDWEIGHTS pipelines (no fp32r
  self-load tax), input DMA halves to 8MB
- attention: 3-way rotating [128,1024] score psum tiles + per-head exp ->
  scalar engine stays saturated (no scores->exp serialization)
- PV col-tiled packed: h0 -> psum partitions 0:64, h1 -> 64:128, one
  [128,1024] accumulator (2 banks)
- softmax denominators: DVE-accumulated den_acc (bf16) + ones-column matmul
  partition-reduce; no ones-column in v, no stag reshape machinery
- compress split into 4 quarter-chunks run at strip boundaries, accumulated
  in SBUF via DVE adds; cg AllGather overlaps strips 2-3
"""
import os
import sys

sys.path.insert(0, "/opt/trn_rl_repo")
os.environ.setdefault("JAX_PLATFORMS", "axon,cpu")

from contextlib import ExitStack

import numpy as np

import concourse.bass as bass
import concourse.mybir as mybir
import concourse.tile as tile
from concourse import bacc
from concourse.bass_utils import run_bass_kernel_spmd
from concourse.masks import make_identity

FP32 = mybir.dt.float32
FP32R = mybir.dt.float32r
BF16 = mybir.dt.bfloat16
AF = mybir.ActivationFunctionType
ALU = mybir.AluOpType

B, T, E = 2, 4096, 1024
LD = 512
D = 64
R = 4
Tc = T // R
NCORES = 8
GROUPS = [[0, 1, 2, 3], [4, 5, 6, 7]]
NEG = -1.0e9


def build_program():
    nc = bacc.Bacc(None, target_bir_lowering=False)

    def inp(name, shape, dt=BF16):
        return nc.declare_dram_parameter(name, list(shape), dt, isOutput=False)

    x2t = inp("x2t", [8, 128, T])            # x[b].T chunks, bf16
    wc = inp("wc", [32, 128, 128])           # compress slice lhsT chunks
    bc = inp("bc", [128, 1], FP32)
    wqkA = inp("wqkA", [4, 128, 128])        # [q_h0/8 | q_h1/8] lhsT chunks
    wqkB = inp("wqkB", [4, 128, 128])        # [k_h0 | k_h1] lhsT chunks
    bqkA = inp("bqkA", [128, 1], FP32)
    bqkB = inp("bqkB", [128, 1], FP32)
    wv = inp("wv", [4, 128, 128])            # [v_h0 | v_h1] rhs chunks
    bvr = inp("bvr", [1, 128])               # [bv_h0 | bv_h1] bias row
    wgqkA = inp("wgqkA", [4, 128, 128])
    wgqkB = inp("wgqkB", [4, 128, 128])
    bgqkA = inp("bgqkA", [128, 1], FP32)
    bgqkB = inp("bgqkB", [128, 1], FP32)
    wgv = inp("wgv", [4, 128, 128])
    bgvr = inp("bgvr", [1, 128])
    wpl = inp("wpl", [4, 128, 129])          # [w_lproj slice | u_l] chunks
    wpg = inp("wpg", [4, 128, 129])
    bple = inp("bple", [1, 129])             # [b_lproj slice, 0]
    bpge = inp("bpge", [1, 129])
    c0h = inp("c0h", [128, 1], FP32)         # 0.5 * gate const
    onesr = inp("onesr", [1, 129])           # ones row (bf16)
    onescb = inp("onescb", [128, 1])         # ones column (bf16)
    repA = inp("repA", [128, 128])           # x4 expander (even 32-blocks)
    repB = inp("repB", [128, 128])
    maskt = inp("maskt", [128, 128])         # strict lower-tri NEG
    out_loc = nc.declare_dram_parameter("out_loc", [T, 128], FP32, isOutput=True)
    out_glob = nc.declare_dram_parameter("out_glob", [T, 128], FP32,
                                         isOutput=True)

    with tile.TileContext(nc) as tc:
      with ExitStack() as top:
        dram = top.enter_context(tc.tile_pool(name="dram", bufs=1, space="DRAM"))
        const = top.enter_context(tc.tile_pool(name="const", bufs=1))
        persist = top.enter_context(tc.tile_pool(name="persist", bufs=1))

        # ---- constants
        ident_b = const.tile([128, 128], BF16, name="ident_b")
        make_identity(nc, ident_b[:])
        mask_tri = const.tile([128, 128], BF16, name="mask_tri")
        nc.sync.dma_start(out=mask_tri[:], in_=maskt[:])
        repA_sb = const.tile([128, 128], BF16, name="repA_sb")
        nc.sync.dma_start(out=repA_sb[:], in_=repA[:])
        repB_sb = const.tile([128, 128], BF16, name="repB_sb")
        nc.sync.dma_start(out=repB_sb[:], in_=repB[:])
        onescb_sb = const.tile([128, 1], BF16, name="onescb_sb")
        nc.sync.dma_start(out=onescb_sb[:], in_=onescb[:])
        ones_f = const.tile([1, 64], FP32, name="ones_f")
        nc.vector.memset(ones_f[:], 1.0)
        ones_bf = const.tile([1, 129], BF16, name="ones_bf")
        nc.sync.dma_start(out=ones_bf[:], in_=onesr[:])
        bple_sb = const.tile([1, 129], BF16, name="bple_sb")
        nc.sync.dma_start(out=bple_sb[:], in_=bple[:])
        bpge_sb = const.tile([1, 129], BF16, name="bpge_sb")
        nc.sync.dma_start(out=bpge_sb[:], in_=bpge[:])
        bvr_sb = const.tile([1, 128], BF16, name="bvr_sb")
        nc.sync.dma_start(out=bvr_sb[:], in_=bvr[:])
        bgvr_sb = const.tile([1, 128], BF16, name="bgvr_sb")
        nc.sync.dma_start(out=bgvr_sb[:], in_=bgvr[:])
        biases = {}
        for nm, src in (("bc", bc), ("c0h", c0h),
                        ("bqkA", bqkA), ("bqkB", bqkB),
                        ("bgqkA", bgqkA), ("bgqkB", bgqkB)):
            t = const.tile([128, 1], FP32, name=f"cb_{nm}")
            nc.sync.dma_start(out=t[:], in_=src[:])
            biases[nm] = t

        # ---- persistent tensors (all bf16)
        qT_l = persist.tile([128, T], BF16, name="qT_l")
        kT_l = persist.tile([128, T], BF16, name="kT_l")
        qT_g = persist.tile([128, Tc], BF16, name="qT_g")
        kT_g = persist.tile([128, Tc], BF16, name="kT_g")
        v_sb_l = [persist.tile([128, 128], BF16, name=f"vsb{i}")
                  for i in range(32)]
        v_sb_g = [persist.tile([128, 128], BF16, name=f"vgsb{i}")
                  for i in range(8)]
        cg_all = [persist.tile([128, Tc], BF16, name=f"cg_all{i}")
                  for i in range(4)]
        cgT32 = persist.tile([128, Tc], FP32, name="cgT32")

        # DRAM bounce tiles for collectives
        cg_contrib = dram.tile([128, Tc], BF16, name="cg_contrib")
        cg_gathered = dram.tile([512, Tc], BF16, name="cg_gathered")
        attl_contrib = [dram.tile([128, 1024], BF16, name=f"alc{s}")
                        for s in range(4)]
        attl_gathered = [dram.tile([512, 1024], BF16, name=f"alg{s}")
                         for s in range(4)]
        attg_contrib = dram.tile([128, Tc], BF16, name="agc")
        attg_gathered = dram.tile([512, Tc], BF16, name="agg")

        ap_pool = top.enter_context(tc.tile_pool(name="attall", bufs=1))
        att_all = [ap_pool.tile([128, T], BF16, name=f"attall{ch}")
                   for ch in range(4)]
        attg_all = [ap_pool.tile([128, Tc], BF16, name=f"attgall{ch}")
                    for ch in range(4)]

        # =========================================== phase AB: x load, qkv, v
        pab = top.enter_context(ExitStack())
        xq_pool = pab.enter_context(tc.tile_pool(name="xq_pool", bufs=1))
        xs_pool = pab.enter_context(tc.tile_pool(name="xs_pool", bufs=8))
        w_pool = pab.enter_context(tc.tile_pool(name="w_pool", bufs=1))
        wcp = pab.enter_context(tc.tile_pool(name="wc_pool", bufs=8))

        x_lo = []
        for cc in range(4):
            t = xq_pool.tile([128, T], BF16, name=f"x2t{cc}")
            nc.scalar.dma_start(out=t[:], in_=x2t[cc])
            x_lo.append(t)

        wqkA_sb, wqkB_sb, wv_sb = [], [], []
        for cc in range(4):
            t = w_pool.tile([128, 128], BF16, name=f"wqkA{cc}")
            nc.sync.dma_start(out=t[:], in_=wqkA[cc])
            wqkA_sb.append(t)
            t = w_pool.tile([128, 128], BF16, name=f"wqkB{cc}")
            nc.sync.dma_start(out=t[:], in_=wqkB[cc])
            wqkB_sb.append(t)
            t = w_pool.tile([128, 128], BF16, name=f"wv{cc}")
            nc.sync.dma_start(out=t[:], in_=wv[cc])
            wv_sb.append(t)

        with ExitStack() as pqkv:
            psAB = pqkv.enter_context(
                tc.tile_pool(name="psAB", bufs=2, space="PSUM"))
            psV = pqkv.enter_context(
                tc.tile_pool(name="psV", bufs=2, space="PSUM"))
            # local q/k: packed tiles -> direct DVE copy
            for (wts, bias_ap, dst) in ((wqkA_sb, biases["bqkA"][:], qT_l),
                                        (wqkB_sb, biases["bqkB"][:], kT_l)):
                for qt in range(8):
                    ps = psAB.tile([128, 512], FP32, name="psAB_t")
                    for cc in range(4):
                        nc.tensor.matmul(
                            ps[:], wts[cc][:],
                            x_lo[cc][:, 512 * qt:512 * qt + 512],
                            start=(cc == 0), stop=(cc == 3))
                    with nc.allow_low_precision(reason="qk bf16"):
                        nc.scalar.activation(dst[:, 512 * qt:512 * qt + 512],
                                             ps[:], AF.Identity, bias=bias_ap)
            # local v: x-stationary, v_sb[t, d] direct
            for kb in range(32):
                ps = psV.tile([128, 128], FP32, name="psV_t")
                for cc in range(4):
                    nc.tensor.matmul(
                        ps[:], x_lo[cc][:, 128 * kb:128 * kb + 128],
                        wv_sb[cc][:], start=(cc == 0), stop=False)
                nc.tensor.matmul(ps[:], ones_bf[:, 0:128], bvr_sb[:],
                                 start=False, stop=True)
                with nc.allow_low_precision(reason="v bf16"):
                    nc.vector.tensor_copy(v_sb_l[kb][:], ps[:])

        # =============================================== attention machinery
        def attention_strip(pools, nkb, q0, qT, kT, v_sb):
            """One 1024-q strip, both heads; returns (hold, den_sb[2])."""
            s_pool, o_pool, p_pool, hold_pool, den_pool, dn_pool = pools
            psum_o = o_pool.tile([128, 1024], FP32, name="ps_o")
            den_acc = [den_pool.tile([128, 1024], BF16, name="den_t")
                       for _ in range(2)]
            pend = None

            def emit_pv(j, ps_list, t0):
                for h in range(2):
                    for qs in (0, 512):
                        lo = max(qs, t0)
                        hi = qs + 512
                        if lo >= hi:
                            continue
                        nc.tensor.matmul(
                            psum_o[64 * h:64 * h + 64, lo:hi],
                            v_sb[j][:, 64 * h:64 * h + 64],
                            ps_list[h][:, lo:hi],
                            start=(j == 0), stop=(j == nkb - 1),
                            skip_group_check=True)

            for j in range(nkb):
                t0 = max(0, 128 * j - q0)
                diag = 128 * j >= q0
                p2 = []
                for h in range(2):
                    ps = s_pool.tile([128, 1024], FP32, name="ps_s")
                    for qs in (0, 512):
                        if qs + 512 <= (t0 // 512) * 512:
                            continue
                        nc.tensor.matmul(
                            ps[:, qs:qs + 512],
                            kT[64 * h:64 * h + 64, 128 * j:128 * j + 128],
                            qT[64 * h:64 * h + 64, q0 + qs:q0 + qs + 512],
                            start=True, stop=True)
                    if diag:
                        nc.tensor.matmul(
                            ps[:, t0:t0 + 128], ident_b[:], mask_tri[:],
                            start=False, stop=True, skip_group_check=True)
                    p = p_pool.tile([128, 1024], BF16, name="p_t")
                    with nc.allow_low_precision(reason="softmax p bf16"):
                        nc.scalar.activation(p[:, t0:1024], ps[:, t0:1024],
                                             AF.Exp)
                    with nc.allow_low_precision(reason="den bf16"):
                        if j == 0:
                            nc.vector.tensor_copy(den_acc[h][:], p[:])
                        else:
                            nc.vector.tensor_add(den_acc[h][:, t0:1024],
                                                 den_acc[h][:, t0:1024],
                                                 p[:, t0:1024])
                    p2.append(p)
                if pend is not None:
                    emit_pv(*pend)
                pend = (j, p2, t0)
            emit_pv(*pend)

            # denominator partition-reduce + 1-lane copies
            den_sb = []
            for h in range(2):
                psd = s_pool.tile([128, 1024], FP32, name="ps_s")
                for qs in (0, 512):
                    nc.tensor.matmul(psd[0:1, qs:qs + 512], onescb_sb[:],
                                     den_acc[h][:, qs:qs + 512],
                                     start=True, stop=True,
                                     skip_group_check=True)
                d = dn_pool.tile([1, 1024], FP32, name="dn_t")
                nc.vector.tensor_copy(d[:], psd[0:1, :])
                den_sb.append(d)
            hold = hold_pool.tile([128, 1024], FP32, name="hold_t")
            nc.vector.tensor_copy(hold[:], psum_o[:])
            return hold, den_sb

        def normalize_strip(pools, hold, den_sb, contrib, gathered):
            s_pool, abp, rpp = pools
            recs = []
            for h in range(2):
                rc32 = rpp.tile([1, 1024], FP32, name="rec32_t")
                nc.vector.reciprocal_approx_fast(out=rc32[:],
                                                 in_=den_sb[h][:])
                recs.append(rc32)
            psw = s_pool.tile([128, 1024], FP32, name="ps_s")
            for h in range(2):
                for qs in (0, 512):
                    nc.tensor.matmul(
                        psw[64 * h:64 * h + 64, qs:qs + 512],
                        ones_f[:], recs[h][:, qs:qs + 512],
                        start=True, stop=True, skip_group_check=True)
            ab = abp.tile([128, 1024], BF16, name="ab_t")
            with nc.allow_low_precision(reason="attnorm bf16"):
                nc.vector.tensor_mul(ab[:], hold[:], psw[:])
            nc.sync.dma_start(out=contrib[:], in_=ab[:])
            nc.gpsimd.collective_compute(
                "AllGather", ALU.bypass, replica_groups=GROUPS,
                ins=[contrib.opt()], outs=[gathered.opt()])

        # =============================================== attention + compress
        with ExitStack() as pc:
            s_pool = pc.enter_context(
                tc.tile_pool(name="s_pool", bufs=3, space="PSUM"))
            o_pool = pc.enter_context(
                tc.tile_pool(name="o_pool", bufs=1, space="PSUM"))
            p_pool = pc.enter_context(tc.tile_pool(name="p_pool", bufs=4))
            hold_pool = pc.enter_context(tc.tile_pool(name="hold", bufs=2))
            den_pool = pc.enter_context(tc.tile_pool(name="den", bufs=4))
            dn_pool = pc.enter_context(tc.tile_pool(name="dn", bufs=2))
            abp = pc.enter_context(tc.tile_pool(name="abp", bufs=2))
            att_pools = (s_pool, o_pool, p_pool, hold_pool, den_pool, dn_pool)
            norm_pools = (s_pool, abp, pc.enter_context(
                tc.tile_pool(name="rpp", bufs=2)))

            # prefetch high x chunks (4..7) as halves; they stream during
            # strips 0-1 and feed compress quarters 2-3
            x_hi = {}
            for cc in range(4, 8):
                h0 = xs_pool.tile([128, 2048], BF16, name="xs_t")
                nc.scalar.dma_start(out=h0[:], in_=x2t[cc][:, 0:2048])
                h1 = xs_pool.tile([128, 2048], BF16, name="xs_t")
                nc.scalar.dma_start(out=h1[:], in_=x2t[cc][:, 2048:4096])
                x_hi[cc] = (h0, h1)

            def compress_quarters(quarters):
                # one quarter: chunk-pair (2cq, 2cq+1) x 4 phases, both halves
                for cq in quarters:
                    psw = s_pool.tile([128, 1024], FP32, name="ps_s")
                    n = 0
                    for cc in (2 * cq, 2 * cq + 1):
                        if cc < 4:
                            srcs = [(x_lo[cc], 0), (x_lo[cc], 2048)]
                        else:
                            srcs = [(x_hi[cc][0], 0), (x_hi[cc][1], 0)]
                        for r in range(4):
                            w = wcp.tile([128, 128], BF16, name="wc_t")
                            nc.sync.dma_start(out=w[:], in_=wc[8 * r + cc])
                            for hf in range(2):
                                xh, off = srcs[hf]
                                nc.tensor.matmul(
                                    psw[:, 512 * hf:512 * hf + 512], w[:],
                                    xh[:, off + r:off + 2048:4],
                                    start=(n == 0), stop=(n == 7))
                            n += 1
                    if cq == 0:
                        nc.vector.tensor_copy(cgT32[:], psw[:])
                    else:
                        nc.vector.tensor_add(cgT32[:], cgT32[:], psw[:])

            holds_l, dens_l = [], []
            for s in range(4):
                h, d = attention_strip(att_pools, 8 * s + 8, 1024 * s,
                                       qT_l, kT_l, v_sb_l)
                normalize_strip(norm_pools, h, d, attl_contrib[s],
                                attl_gathered[s])
                for ch in range(4):
                    nc.sync.dma_start(
                        out=att_all[ch][:, 1024 * s:1024 * s + 1024],
                        in_=attl_gathered[s][128 * ch:128 * ch + 128, :])
                if s == 0:
                    compress_quarters([0, 1])
                elif s == 1:
                    compress_quarters([2, 3])
                    # finalize cg: bias + bf16 cast, AllGather
                    cg_bf = abp.tile([128, 1024], BF16, name="ab_t")
                    with nc.allow_low_precision(reason="cg bf16"):
                        nc.scalar.activation(cg_bf[:], cgT32[:], AF.Identity,
                                             bias=biases["bc"][:])
                    nc.sync.dma_start(out=cg_contrib[:], in_=cg_bf[:])
                    nc.gpsimd.collective_compute(
                        "AllGather", ALU.bypass, replica_groups=GROUPS,
                        ins=[cg_contrib.opt()], outs=[cg_gathered.opt()])
                    for i in range(4):
                        nc.sync.dma_start(
                            out=cg_all[i][:],
                            in_=cg_gathered[128 * i:128 * i + 128, :])
                elif s == 2:
                    # global qkv + v from gathered cg
                    with ExitStack() as pg:
                        wgp = pg.enter_context(
                            tc.tile_pool(name="wg_pool", bufs=1))
                        wgqkA_sb, wgqkB_sb, wgv_sb = [], [], []
                        for cc in range(4):
                            t = wgp.tile([128, 128], BF16, name=f"wgqkA{cc}")
                            nc.sync.dma_start(out=t[:], in_=wgqkA[cc])
                            wgqkA_sb.append(t)
                            t = wgp.tile([128, 128], BF16, name=f"wgqkB{cc}")
                            nc.sync.dma_start(out=t[:], in_=wgqkB[cc])
                            wgqkB_sb.append(t)
                            t = wgp.tile([128, 128], BF16, name=f"wgv{cc}")
                            nc.sync.dma_start(out=t[:], in_=wgv[cc])
                            wgv_sb.append(t)
                        for (wts, bias_ap, dst) in (
                                (wgqkA_sb, biases["bgqkA"][:], qT_g),
                                (wgqkB_sb, biases["bgqkB"][:], kT_g)):
                            for qt in range(2):
                                psw = s_pool.tile([128, 1024], FP32,
                                                  name="ps_s")
                                ps = psw[:, 0:512]
                                for cc in range(4):
                                    nc.tensor.matmul(
                                        ps, wts[cc][:],
                                        cg_all[cc][:, 512 * qt:512 * qt + 512],
                                        start=(cc == 0), stop=(cc == 3))
                                with nc.allow_low_precision(reason="gqk bf16"):
                                    nc.scalar.activation(
                                        dst[:, 512 * qt:512 * qt + 512],
                                        ps, AF.Identity, bias=bias_ap)
                        for kb in range(8):
                            psw = s_pool.tile([128, 1024], FP32, name="ps_s")
                            ps = psw[:, 0:128]
                            for cc in range(4):
                                nc.tensor.matmul(
                                    ps, cg_all[cc][:, 128 * kb:128 * kb + 128],
                                    wgv_sb[cc][:], start=(cc == 0), stop=False)
                            nc.tensor.matmul(ps, ones_bf[:, 0:128],
                                             bgvr_sb[:],
                                             start=False, stop=True)
                            with nc.allow_low_precision(reason="gv bf16"):
                                nc.vector.tensor_copy(v_sb_g[kb][:], ps)
                    # global attention here so its AllGather overlaps strip 3
                    gh, gd = attention_strip(att_pools, 8, 0,
                                             qT_g, kT_g, v_sb_g)
                    normalize_strip(norm_pools, gh, gd, attg_contrib,
                                    attg_gathered)
                    for ch in range(4):
                        nc.sync.dma_start(
                            out=attg_all[ch][:],
                            in_=attg_gathered[128 * ch:128 * ch + 128, :])

        pab.close()

        # =============================================== proj + gate + out
        with ExitStack() as pd:
            wpp = pd.enter_context(tc.tile_pool(name="wp_pool", bufs=1))
            psP = pd.enter_context(tc.tile_pool(name="psP", bufs=2,
                                                space="PSUM"))
            psE = pd.enter_context(tc.tile_pool(name="psE", bufs=2,
                                                space="PSUM"))
            psD = pd.enter_context(tc.tile_pool(name="psD", bufs=1,
                                                space="PSUM"))
            gp = pd.enter_context(tc.tile_pool(name="g_pool", bufs=1))
            outp = pd.enter_context(tc.tile_pool(name="out_pool", bufs=4))
            stp = pd.enter_context(tc.tile_pool(name="stage_pool", bufs=1))

            wpl_sb, wpg_sb = [], []
            for ch in range(4):
                t = wpp.tile([128, 129], BF16, name=f"wpl{ch}")
                nc.sync.dma_start(out=t[:], in_=wpl[ch])
                wpl_sb.append(t)
                t = wpp.tile([128, 129], BF16, name=f"wpg{ch}")
                nc.sync.dma_start(out=t[:], in_=wpg[ch])
                wpg_sb.append(t)

            # global proj first (its gate column feeds the local gate)
            dg_sb = gp.tile([128, 8], BF16, name="dg_sb")
            gstage = []
            for tbg in range(8):
                ps = psP.tile([128, 129], FP32, name="psPg_t")
                for ch in range(4):
                    nc.tensor.matmul(
                        ps[:], attg_all[ch][:, 128 * tbg:128 * tbg + 128],
                        wpg_sb[ch][:], start=(ch == 0), stop=False)
                nc.tensor.matmul(ps[:], ones_bf[:, 0:128], bpge_sb[:],
                                 start=False, stop=True)
                with nc.allow_low_precision(reason="gate logit bf16"):
                    nc.vector.tensor_copy(dg_sb[:, tbg:tbg + 1],
                                          ps[:, 128:129])
                gt = gp.tile([128, 128], BF16, name=f"gst{tbg}")
                with nc.allow_low_precision(reason="gproj bf16 for expand"):
                    nc.vector.tensor_copy(gt[:], ps[:, 0:128])
                gstage.append(gt)

            # expand dg x4 into natural token blocks: dgx [128, 32]
            ps_dgx = psD.tile([128, 32], FP32, name="ps_dgx")
            for tb in range(32):
                base = 64 * ((tb % 4) // 2)
                rep = repA_sb if tb % 2 == 0 else repB_sb
                nc.tensor.matmul(ps_dgx[:, tb:tb + 1],
                                 rep[base:base + 64, :],
                                 dg_sb[base:base + 64, tb // 4:tb // 4 + 1],
                                 start=True, stop=True, skip_group_check=True)

            # local proj: psum -> outstage + dl column
            dl_sb = gp.tile([128, 32], FP32, name="dl_sb")
            outst = []
            for tb in range(32):
                ps = psP.tile([128, 129], FP32, name="psPl_t")
                for ch in range(4):
                    nc.tensor.matmul(
                        ps[:], att_all[ch][:, 128 * tb:128 * tb + 128],
                        wpl_sb[ch][:], start=(ch == 0), stop=False)
                nc.tensor.matmul(ps[:], ones_bf[:, 0:128], bple_sb[:],
                                 start=False, stop=True)
                nc.vector.tensor_copy(dl_sb[:, tb:tb + 1], ps[:, 128:129])
                ot = stp.tile([128, 128], FP32, name=f"outst{tb}")
                nc.scalar.activation(ot[:], ps[:, 0:128], AF.Copy)
                outst.append(ot)

            # gate: tanh(0.5*(dl+dgx) + 0.5*c0)
            dsum = gp.tile([128, 32], FP32, name="dsum")
            nc.vector.tensor_add(dsum[:], dl_sb[:], ps_dgx[:])
            tanh_sb = gp.tile([128, 32], FP32, name="tanh_sb")
            nc.scalar.activation(tanh_sb[:], dsum[:], AF.Tanh,
                                 scale=0.5, bias=biases["c0h"][:])
            g0 = gp.tile([128, 32], FP32, name="g0")
            g1 = gp.tile([128, 32], FP32, name="g1")
            nc.vector.tensor_scalar(g0[:], tanh_sb[:], 0.5, 0.5,
                                    ALU.mult, ALU.add)
            nc.vector.tensor_scalar(g1[:], tanh_sb[:], -0.5, 0.5,
                                    ALU.mult, ALU.add)

            for tb in range(32):
                o = outp.tile([128, 128], FP32, name="outl")
                nc.vector.tensor_scalar_mul(o[:], outst[tb][:],
                                            g0[:, tb:tb + 1])
                nc.sync.dma_start(out=out_loc[128 * tb:128 * tb + 128, :],
                                  in_=o[:])
            for tb in range(32):
                ps = psE.tile([128, 128], FP32, name="psE_t")
                base = 64 * ((tb % 4) // 2)
                rep = repA_sb if tb % 2 == 0 else repB_sb
                nc.tensor.matmul(ps[:], rep[base:base + 64, :],
                                 gstage[tb // 4][base:base + 64, :],
                                 start=True, stop=True)
                o = outp.tile([128, 128], FP32, name="outg")
                nc.vector.tensor_scalar_mul(o[:], ps[:], g1[:, tb:tb + 1])
                nc.sync.dma_start(out=out_glob[128 * tb:128 * tb + 128, :],
                                  in_=o[:])

    nc.finalize()
    return nc


# ---------------------------------------------------------------------------
# Host side
# ---------------------------------------------------------------------------

_NC_CACHE = []


def _get_program():
    if not _NC_CACHE:
        _NC_CACHE.append(build_program())
    return _NC_CACHE[0]


def _prep_inputs(x, w_lqkv, b_lqkv, w_gqkv, b_gqkv, w_comp, b_comp,
                 w_lproj, b_lproj, w_gproj, b_gproj, w_gate, b_gate):
    import ml_dtypes
    f32 = np.float32
    bf16 = ml_dtypes.bfloat16
    wd = (w_gate[:, 0] - w_gate[:, 1]).astype(f32)
    u_l = (w_lproj @ wd[:LD]).astype(f32)
    u_g = (w_gproj @ wd[LD:]).astype(f32)
    c0 = float(b_lproj @ wd[:LD] + b_gproj @ wd[LD:] + b_gate[0] - b_gate[1])

    mask_tri = np.where(np.arange(128)[None, :] >= np.arange(128)[:, None],
                        0.0, NEG).astype(f32)
    e0 = np.zeros((64, 128), f32)
    e0[np.arange(128) // 4, np.arange(128)] = 1.0
    e1 = np.zeros((64, 128), f32)
    e1[32 + np.arange(128) // 4, np.arange(128)] = 1.0
    repA_ = np.concatenate([e0, e0], axis=0)
    repB_ = np.concatenate([e1, e1], axis=0)
    sel2_ = np.zeros((2, 128), f32)
    sel2_[0, 0:64] = 1.0
    sel2_[1, 64:128] = 1.0

    def qk_packed(wqkv, bqkv, ha, hb):
        wA = np.concatenate([wqkv[:, D * ha:D * ha + D] / 8.0,
                             wqkv[:, D * hb:D * hb + D] / 8.0], axis=1)
        bA = np.concatenate([bqkv[D * ha:D * ha + D] / 8.0,
                             bqkv[D * hb:D * hb + D] / 8.0])
        wB = np.concatenate([wqkv[:, LD + D * ha:LD + D * ha + D],
                             wqkv[:, LD + D * hb:LD + D * hb + D]], axis=1)
        bB = np.concatenate([bqkv[LD + D * ha:LD + D * ha + D],
                             bqkv[LD + D * hb:LD + D * hb + D]])
        return (wA.reshape(4, 128, 128), bA.astype(f32).reshape(128, 1),
                wB.reshape(4, 128, 128), bB.astype(f32).reshape(128, 1))

    def v_packed(wqkv, bqkv, ha, hb):
        wv_ = np.concatenate([wqkv[:, 2 * LD + D * ha:2 * LD + D * ha + D],
                              wqkv[:, 2 * LD + D * hb:2 * LD + D * hb + D]],
                             axis=1)
        bv_ = np.concatenate([bqkv[2 * LD + D * ha:2 * LD + D * ha + D],
                              bqkv[2 * LD + D * hb:2 * LD + D * hb + D]])
        return wv_.reshape(4, 128, 128), bv_.astype(f32).reshape(1, 128)

    in_maps = []
    for core in range(NCORES):
        b_idx, g = core // 4, core % 4
        ha, hb = 2 * g, 2 * g + 1
        cs = slice(128 * g, 128 * g + 128)

        x2t_ = np.ascontiguousarray(x[b_idx].T).reshape(8, 128, T)
        wc_s = np.ascontiguousarray(
            w_comp[:, LD + 128 * g:LD + 128 * g + 128]).reshape(32, 128, 128)
        bc_s = b_comp[LD + 128 * g:LD + 128 * g + 128].astype(f32) \
            .reshape(128, 1)

        wqkA_, bqkA_, wqkB_, bqkB_ = qk_packed(w_lqkv, b_lqkv, ha, hb)
        wv_, bvr_ = v_packed(w_lqkv, b_lqkv, ha, hb)
        wgqkA_, bgqkA_, wgqkB_, bgqkB_ = qk_packed(w_gqkv, b_gqkv, ha, hb)
        wgv_, bgvr_ = v_packed(w_gqkv, b_gqkv, ha, hb)

        wpl_ = np.concatenate([w_lproj[:, cs], u_l[:, None]],
                              axis=1).reshape(4, 128, 129)
        wpg_ = np.concatenate([w_gproj[:, cs], u_g[:, None]],
                              axis=1).reshape(4, 128, 129)
        bple_ = np.concatenate([b_lproj[cs], [0.0]]).astype(f32) \
            .reshape(1, 129)
        bpge_ = np.concatenate([b_gproj[cs], [0.0]]).astype(f32) \
            .reshape(1, 129)

        m = {
            "x2t": x2t_, "wc": wc_s, "bc": bc_s,
            "wqkA": wqkA_, "bqkA": bqkA_, "wqkB": wqkB_, "bqkB": bqkB_,
            "wv": wv_, "bvr": bvr_,
            "wgqkA": wgqkA_, "bgqkA": bgqkA_, "wgqkB": wgqkB_,
            "bgqkB": bgqkB_, "wgv": wgv_, "bgvr": bgvr_,
            "wpl": wpl_, "wpg": wpg_, "bple": bple_, "bpge": bpge_,
            "c0h": np.full((128, 1), 0.5 * c0, f32),
            "onesr": np.ones((1, 129), f32),
            "onescb": np.ones((128, 1), f32),
            "repA": repA_, "repB": repB_, "maskt": mask_tri,
        }
        for k in ("x2t", "wc", "wqkA", "wqkB", "wv", "bvr", "wgqkA", "wgqkB",
                  "wgv", "bgvr", "wpl", "wpg", "bple", "bpge", "onesr",
                  "onescb", "repA", "repB", "maskt"):
            m[k] = m[k].astype(bf16)
        in_maps.append(m)
    return in_maps


def _run(in_maps, trace=False):
    nc = _get_program()
    return run_bass_kernel_spmd(nc, in_maps, list(range(NCORES)), trace=trace)


def assemble(results):
    out = np.empty((B, T, E), np.float32)
    for core in range(NCORES):
        b_idx, g = core // 4, core % 4
        out[b_idx, :, 128 * g:128 * g + 128] = results[core]["out_loc"]
        out[b_idx, :, LD + 128 * g:LD + 128 * g + 128] = \
            results[core]["out_glob"]
    return out


def kernel(**inputs):
    in_maps = _prep_inputs(**inputs)
    res = _run(in_maps)
    return assemble(res.results)


def kernel_traced(**inputs):
    in_maps = _prep_inputs(**inputs)
    res = _run(in_maps, trace=True)
    return assemble(res.results), res

